# revision 45
# baseline (speedup 1.0000x reference)
"""Multi-head attention (B=4, T=2048, D=1024, H=16) on 8 TRN2 NeuronCores.

Sharding: batch x head-half (4 batches x 2 halves of 8 heads = 8 cores).
Each core projects Q/K/V for its 8 heads over the full 2048 tokens, runs
attention, and computes partial output projections against its half of Wo.
The tensor-parallel FC "all-reduce" is a host-side sum of the partials.

Per-core program (all matmul inputs bf16, fp32 PSUM accumulation):
  - Scores S = K_blk @ Q^T land as [128 ktok, 1024 q] PSUM tiles; one exp
    per tile (scalar engine) writes P directly as bf16. The exp stream is
    the pacer (~266us); emission follows a global entry cursor: the score
    matmuls for entry i are gated on exp(i-2) freeing a PSUM slot, and the
    PV steps trail two entries behind on the same gates.
  - PV is kb-major and output-stationary: 8 PSUM accumulators [128 q, 65]
    (ones-augmented V gives the softmax denominator in col 64) accumulate
    one k-block right after its exp, so PV(n) completes with exp(n,15).
  - Normalization (DVE reciprocal + scalar multiply) writes two heads of a
    block side-by-side into [128 q, 128 hd] pair tiles; a DMA-engine XBAR
    transpose moves them into head-major otT with zero PE cost.
  - The FC accumulates all 4 head-pair blocks in PSUM per token block; the
    qh=0 half runs inside the last exp window, the qh=1 half trails PV(15).
  - Bootstrap: wq rides the gpsimd queue and the first xq eighths ride the
    small c-slots so the first exp fires ~9us in; the first four score
    tiles are split into 512-wide halves to keep the exp cursor moving
    while projections catch up. xk quarters hand their slots straight to
    xv (block-1 K/Q re-load as eighths in w1/w2, like the block-2/3 fills).

Host side: transposes inputs to feature-major bf16, slices weights per
head-half, runs SPMD on 8 cores, sums the two partial y per batch, and
adds the exact (bv @ Wo.T + bo) bias (attention rows sum to 1 so the value
bias passes through; bq/bk are zero in this problem).
"""
import numpy as np
from contextlib import ExitStack

import ml_dtypes

import concourse.bass as bass
import concourse.tile as tile
from concourse import bacc, mybir
from concourse.bass_utils import run_bass_kernel_spmd

F32 = mybir.dt.float32
BF16 = mybir.dt.bfloat16
NPBF16 = ml_dtypes.bfloat16

B = 4
T = 2048
D = 1024
H = 16
DK = 64
NCORES = 8
HLOC = 8           # heads per core
DHALF = 512        # hd dims per core
NKB = T // 128     # 16 key blocks
EXP_SCALE = 1.0 / np.sqrt(DK)


def _emit(nc):
    xq = nc.dram_tensor("xq", [D, T], BF16, kind="ExternalInput").ap()   # query^T
    xk = nc.dram_tensor("xk", [D, T], BF16, kind="ExternalInput").ap()   # key^T
    xv = nc.dram_tensor("xv", [D, T], BF16, kind="ExternalInput").ap()   # value^T
    wq = nc.dram_tensor("wq", [D, DHALF], BF16, kind="ExternalInput").ap()
    wk = nc.dram_tensor("wk", [D, DHALF], BF16, kind="ExternalInput").ap()
    wv = nc.dram_tensor("wv", [D, DHALF], BF16, kind="ExternalInput").ap()
    wo = nc.dram_tensor("wo", [DHALF, D], BF16, kind="ExternalInput").ap()
    y = nc.dram_tensor("y", [T, D], BF16, kind="ExternalOutput").ap()  # partial

    with tile.TileContext(nc) as tc, ExitStack() as ctx:
        res = ctx.enter_context(tc.tile_pool(name="res", bufs=1))
        otT = res.tile([128, 4, T], BF16)      # normalized attention out^T
        vaug = res.tile([128, NKB, HLOC, DK + 1], BF16)
        wot = res.tile([128, 4, D], BF16)      # Wo^T slice [(ki p) m -> p ki m]
        zz = res.tile([1, 512], BF16)          # zero row for PSUM-bank zeroing
        nc.vector.memset(vaug[:, :, :, DK:DK + 1], 1.0)
        nc.vector.memset(zz[:], 0.0)

        # K^T / Q^T blocks [128 hd, 2048 tok], two rotating slots per tag.
        ktq = ctx.enter_context(tc.tile_pool(name="ktq", bufs=2))
        kts, qts = {}, {}

        wst = ctx.enter_context(tc.tile_pool(name="wst", bufs=1))
        wk_s = wst.tile([128, 8, DHALF], BF16, name="wk_s", tag="wk")
        wq_s = wst.tile([128, 8, DHALF], BF16, name="wq_s", tag="wq")
        wv_s = wst.tile([128, 8, DHALF], BF16, name="wv_s", tag="wv")

        # Input staging: four quarter slots [128, 8, 512] carry xk then xv
        # (xv pinned for the per-head V quanta); slot c is a 2-deep eighth
        # slab [128, 8, 256] for all xq traffic and the K/Q re-load fills.
        xst = ctx.enter_context(tc.tile_pool(name="xst", bufs=1))

        # PSUM (8 banks): score/FC tiles [128,1024] x2 (4 banks), PV
        # accumulators [128,4,128] x2 (2), projection chains [128,512] x2.
        bigp = ctx.enter_context(tc.tile_pool(name="bigp", bufs=2, space="PSUM"))
        pvp = ctx.enter_context(tc.tile_pool(name="pvp", bufs=1, space="PSUM"))
        pva = pvp.tile([128, 4, 128], F32, name="pva", tag="pva")
        pvb = pvp.tile([128, 4, 128], F32, name="pvb", tag="pvb")
        smp = ctx.enter_context(tc.tile_pool(name="smp", bufs=2, space="PSUM"))

        ptp = ctx.enter_context(tc.tile_pool(name="ptp", bufs=14))  # P ring
        nrm = ctx.enter_context(tc.tile_pool(name="nrm", bufs=1))   # recip
        prp = ctx.enter_context(tc.tile_pool(name="prp", bufs=2))   # pair tiles
        evp = ctx.enter_context(tc.tile_pool(name="evp", bufs=1))   # fc evict

        def load_quarter(src, i, slot, eng):
            xs = xst.tile([128, 8, 512], BF16, name=f"x_{slot}", tag=slot)
            eng.dma_start(
                xs, src[:, i * 512:(i + 1) * 512]
                .rearrange("(ki p) t -> p ki t", p=128))
            return xs

        def load_eighth(src, e, eng):
            xs = xst.tile([128, 8, 256], BF16, name="x_c", tag="c", bufs=2)
            eng.dma_start(
                xs, src[:, e * 256:(e + 1) * 256]
                .rearrange("(ki p) t -> p ki t", p=128))
            return xs

        written = {}   # (which, blk) -> set of written 128-col chunks

        def kq_chain(w_s, blk, xs, xcol, width, dst, dstcol):
            """8-ki projection chain xs[:,:,xcol:xcol+width] -> dst cols."""
            which = "k" if w_s is wk_s else "q"
            written.setdefault((which, blk), set()).update(
                range(dstcol // 128, (dstcol + width) // 128))
            ps = smp.tile([128, 512], F32, name="pps", tag="sm")
            for ki in range(8):
                nc.tensor.matmul(
                    ps[:, 0:width],
                    lhsT=w_s[:, ki, blk * 128:(blk + 1) * 128],
                    rhs=xs[:, ki, xcol:xcol + width],
                    start=(ki == 0), stop=(ki == 7))
            nc.vector.tensor_copy(dst[:, dstcol:dstcol + width], ps[:, 0:width])

        def blk_tile(which, blk):
            tiles = kts if which == "k" else qts
            if blk not in tiles:
                tiles[blk] = ktq.tile([128, T], BF16,
                                      name=f"{which}t{blk}",
                                      tag=which)
            return tiles[blk]

        # K/Q re-load fills via the 2-deep c slot (blocks 1-3).
        fill_q = []

        def fill_dma(blk, which, e):
            src = xk if which == "k" else xq
            xs = load_eighth(src, e, nc.sync)
            fill_q.append((blk, which, e, xs))

        def fill_chain():
            blk, which, e, xs = fill_q.pop(0)
            w_s = wk_s if which == "k" else wq_s
            kq_chain(w_s, blk, xs, 0, 256, blk_tile(which, blk), e * 256)

        xv_q = [None] * 4

        def v_subq(h, qi):
            """V projection for head h, token blocks 4*qi..4*qi+4."""
            xs = xv_q[qi]
            for tb in range(4 * qi, 4 * qi + 4):
                ps = smp.tile([128, 512], F32, name="vps", tag="sm")
                for ki in range(8):
                    nc.tensor.matmul(
                        ps[:, 0:DK],
                        lhsT=xs[:, ki, (tb % 4) * 128:(tb % 4 + 1) * 128],
                        rhs=wv_s[:, ki, h * DK:(h + 1) * DK],
                        start=(ki == 0), stop=(ki == 7))
                nc.vector.tensor_copy(vaug[:, tb, h, 0:DK], ps[:, 0:DK])

        pts = {}

        def score_tile(n, kb, half=None):
            """Score matmuls + exp for tile (n, kb); half 0/1 = 512-wide."""
            h, qh = divmod(n, 2)
            blk, po = h // 2, (h % 2) * 64
            ktb, qtb = kts[blk], qts[blk]
            if (n, kb) not in pts:
                pts[(n, kb)] = ptp.tile([128, 1024], BF16,
                                        name="pt", tag="pt")
            pt = pts[(n, kb)]
            cs = (0, 1) if half is None else (half,)
            assert kb in written[("k", blk)], f"S({n},{kb}): kt{blk} race"
            for c in cs:
                need_q = range(8 * qh + 4 * c, 8 * qh + 4 * c + 4)
                assert written[("q", blk)].issuperset(need_q), \
                    f"S({n},{kb},{c}): qt{blk} race"
            st = bigp.tile([128, 1024], F32, name="st", tag="big")
            for c in cs:
                nc.tensor.matmul(
                    st[:, c * 512:(c + 1) * 512],
                    lhsT=ktb[po:po + 64, kb * 128:(kb + 1) * 128],
                    rhs=qtb[po:po + 64,
                            qh * 1024 + c * 512:qh * 1024 + (c + 1) * 512],
                    start=True, stop=True)
            c0, width = cs[0] * 512, len(cs) * 512
            nc.scalar.activation(
                pt[:, c0:c0 + width], st[:, c0:c0 + width],
                mybir.ActivationFunctionType.Exp, scale=EXP_SCALE)

        def pv_zero(accs=(0, 1)):
            """Zero PV accumulator banks with one start=True matmul each.
            PSUM zero-region granularity is the whole 2KB bank, so per-slot
            start=True flags from the interleaved qb series would re-mark
            each other's bytes and drop contributions; an explicit full-bank
            start + accumulate-only steps is the safe pattern."""
            for a in accs:
                acc = (pva, pvb)[a]
                nc.tensor.matmul(
                    acc.rearrange("p a b -> p (a b)"),
                    lhsT=zz[:, 0:128], rhs=zz[:, 0:512],
                    start=True, stop=False)

        def pv_step(n, kb, qbs):
            """PV accumulation of k-block kb for window n, given q-blocks."""
            h = n // 2
            if kb == 0 and n > 0 and qbs[0] == 0:
                pv_zero((0, 1))   # re-zero banks; norms(n-1) reads are done
            if qbs[-1] == 7:      # last reader of this pt tile
                pt = pts.pop((n, kb))
            else:
                pt = pts[(n, kb)]
            for qb in qbs:
                acc = (pva, pvb)[qb // 4]
                nc.tensor.matmul(
                    acc[:, qb % 4, 0:DK + 1],
                    lhsT=pt[:, qb * 128:(qb + 1) * 128],
                    rhs=vaug[:, kb, h, :],
                    start=False, stop=(kb == NKB - 1))

        pairs = {}

        def norms(n):
            """Normalize window n into pair tiles [128 q, 128 hd]. One
            batched reciprocal per bank (4 denominators), then 4 scalar
            multiplies; each bank is re-zeroed for the next window as soon
            as its four reads are done so the PE stall at the window
            boundary is short."""
            h, qh = divmod(n, 2)
            co = (h % 2) * 64
            for a, acc in enumerate((pva, pvb)):
                rd = nrm.tile([128, 4], F32, name="rd", tag="rd", bufs=2)
                nc.vector.reciprocal(rd[:], acc[:, :, DK:DK + 1])
                for s in range(4):
                    qb = 4 * a + s
                    if (qh, qb) not in pairs:
                        pairs[(qh, qb)] = prp.tile(
                            [128, 128], BF16, name=f"pr{qh}_{qb}",
                            tag=f"pr{qh}{qb}")
                    nc.vector.tensor_scalar_mul(
                        pairs[(qh, qb)][:, co:co + DK], acc[:, s, 0:DK],
                        rd[:, s:s + 1])

        def transposes(blk, qh, engs):
            """XBAR DMA transpose pair tiles -> otT for (blk, qh)."""
            for qb in range(8):
                pr = pairs.pop((qh, qb))
                engs[qb % len(engs)].dma_start_transpose(
                    otT[:, blk, qh * 1024 + qb * 128:qh * 1024 + (qb + 1) * 128],
                    pr)

        def fc_tb(tb, eng, use_smp=False):
            """Output projection for one token block (all 4 ki accumulated).
            Evictions rotate through dead staging slots. use_smp runs the
            two 512-col halves through the small PSUM pool so the score
            pipeline keeps both bigp slots during the last exp window."""
            ev_slots = ["s0", "s1", "s2", "s3", "wk", "wq", "c", "ev"]
            slot = ev_slots[tb % 8]
            pool = {"wk": wst, "wq": wst, "ev": evp}.get(slot, xst)
            ev = pool.tile([128, 1024], BF16, name="ev", tag=slot,
                           bufs=2 if slot == "c" else 1)
            if use_smp:
                for c in range(2):
                    fp = smp.tile([128, 512], F32, name="fch", tag="sm")
                    for ki in range(4):
                        nc.tensor.matmul(
                            fp[:],
                            lhsT=otT[:, ki, tb * 128:(tb + 1) * 128],
                            rhs=wot[:, ki, c * 512:(c + 1) * 512],
                            start=(ki == 0), stop=(ki == 3))
                    nc.vector.tensor_copy(ev[:, c * 512:(c + 1) * 512], fp[:])
            else:
                fp = bigp.tile([128, 1024], F32, name="fcp", tag="big")
                for c in range(2):
                    for ki in range(4):
                        nc.tensor.matmul(
                            fp[:, c * 512:(c + 1) * 512],
                            lhsT=otT[:, ki, tb * 128:(tb + 1) * 128],
                            rhs=wot[:, ki, c * 512:(c + 1) * 512],
                            start=(ki == 0), stop=(ki == 3))
                    nc.vector.tensor_copy(
                        ev[:, c * 512:(c + 1) * 512],
                        fp[:, c * 512:(c + 1) * 512])
                    eng.dma_start(
                        y[tb * 128:(tb + 1) * 128, c * 512:(c + 1) * 512],
                        ev[:, c * 512:(c + 1) * 512])
                return
            eng.dma_start(y[tb * 128:(tb + 1) * 128, :], ev[:])

        # ---- bootstrap ----
        # Queues: sync=SP, scalar=ACT, gpsimd=Pool(SWDGE). wq rides gpsimd,
        # wk + the xq eighths ride scalar, xk/xv quarters ride sync/gpsimd.
        nc.gpsimd.dma_start(wq_s[:, :, 0:256],
                            wq[:, 0:256].rearrange("(ki p) m -> p ki m", p=128))
        xqe = [load_eighth(xq, e, nc.scalar) for e in range(2)]
        nc.scalar.dma_start(wk_s[:, :, 0:256],
                            wk[:, 0:256].rearrange("(ki p) m -> p ki m", p=128))
        xk_q = [load_quarter(xk, 0, "s0", nc.sync)]

        # PE p-state warmup: the cost model runs the PE at 1.2 GHz until it
        # has been continuously busy for 3us. Junk matmuls bridge the DMA
        # wait so the real chains start at full speed (2.4 GHz).
        pv_zero()
        for j in range(5):
            jt = bigp.tile([128, 1024], F32, name="jnk", tag="big")
            nc.tensor.matmul(jt[:, 0:512], lhsT=zz[:, 0:128],
                             rhs=zz[:, 0:512], start=True, stop=True)

        blk_tile("k", 0)
        blk_tile("q", 0)

        # First chains: qt0 tokens 0:512 from the c-slot eighths (their DMAs
        # land first), then kt0 tokens 0:512 in 128-wide slices.
        kq_chain(wq_s, 0, xqe[0], 0, 256, qts[0], 0)
        kq_chain(wq_s, 0, xqe[1], 0, 256, qts[0], 256)
        for s in range(4):
            kq_chain(wk_s, 0, xk_q[0], s * 128, 128, kts[0], s * 128)
        nc.gpsimd.dma_start(wq_s[:, :, 256:512],
                            wq[:, 256:512].rearrange("(ki p) m -> p ki m", p=128))
        xqe += [load_eighth(xq, e, nc.scalar) for e in range(2, 4)]
        nc.scalar.dma_start(wk_s[:, :, 256:512],
                            wk[:, 256:512].rearrange("(ki p) m -> p ki m", p=128))
        nc.gpsimd.dma_start(wv_s, wv.rearrange("(ki p) m -> p ki m", p=128))
        xk_q.append(load_quarter(xk, 1, "s1", nc.sync))

        # First four score tiles in 512-wide halves to start the exp stream
        # while qt0's second half is still projecting.
        for kb in range(4):
            score_tile(0, kb, half=0)
        kq_chain(wq_s, 0, xqe[2], 0, 256, qts[0], 512)
        kq_chain(wq_s, 0, xqe[3], 0, 256, qts[0], 768)
        for kb in range(4):
            score_tile(0, kb, half=1)

        # ---- per-window work queues ----
        WORK = {w: [] for w in range(16)}

        def _xkq(i):
            def f():
                xk_q.append(load_quarter(xk, i, f"s{i}", nc.sync))
            return f

        def _ktchain(blk, i, dstcol):
            return lambda: kq_chain(wk_s, blk, xk_q[i], 0, 512,
                                    blk_tile("k", blk), dstcol)

        def _xqe(e):
            def f():
                xqe.append(load_eighth(xq, e, nc.scalar))
            return f

        def _qtchain(blk, e, dstcol):
            return lambda: kq_chain(wq_s, blk, xqe[e], 0, 256,
                                    blk_tile("q", blk), dstcol)

        def _xvq(i, eng):
            def f():
                xv_q[i] = load_quarter(xv, i, f"s{i}", eng)
            return f

        vdone = [0] * 8

        def _vsub(h, qi):
            def f():
                v_subq(h, qi)
                vdone[h] += 1
            return f

        def _wot():
            nc.gpsimd.dma_start(wot, wo.rearrange("(ki p) m -> p ki m", p=128))

        FILLS = ([(1, "k", e) for e in range(8)] +
                 [(1, "q", e) for e in range(8)] +
                 [(2, "k", e) for e in range(8)] +
                 [(2, "q", e) for e in range(8)] +
                 [(3, "k", e) for e in range(8)] +
                 [(3, "q", e) for e in range(8)])
        fill_i = [0]

        def _fill():
            blk, which, e = FILLS[fill_i[0]]
            fill_i[0] += 1
            fill_dma(blk, which, e)

        def _fc_half(tb, c, eng):
            """One 512-col half of FC for tb through the small PSUM pool;
            eviction + store on the c=1 half."""
            def f():
                ev_slots = ["s0", "s1", "s2", "s3", "wk", "wq", "c", "ev"]
                slot = ev_slots[tb % 8]
                pool = {"wk": wst, "wq": wst, "ev": evp}.get(slot, xst)
                if tb not in fc_ev:
                    fc_ev[tb] = pool.tile([128, 1024], BF16, name="ev",
                                          tag=slot,
                                          bufs=2 if slot == "c" else 1)
                ev = fc_ev[tb]
                fp = smp.tile([128, 512], F32, name="fch", tag="sm")
                for ki in range(4):
                    nc.tensor.matmul(
                        fp[:],
                        lhsT=otT[:, ki, tb * 128:(tb + 1) * 128],
                        rhs=wot[:, ki, c * 512:(c + 1) * 512],
                        start=(ki == 0), stop=(ki == 3))
                nc.vector.tensor_copy(ev[:, c * 512:(c + 1) * 512], fp[:])
                if c == 1:
                    eng.dma_start(y[tb * 128:(tb + 1) * 128, :], ev[:])
            return f

        fc_ev = {}

        # w0: rest of kt0 (tokens 512:2048 from xk quarters as they land),
        # qt0 tokens 1024:2048 (xq eighths 4-7), xv quarter loads, V0.
        # Order matters: the kt0 chain feeding score tiles (0, 4e..4e+4)
        # must be EMITTED before those score tiles (drain index < 4 * e + 4),
        # else the tile framework sees the read first (race -> garbage).
        WORK[0] = [
            _ktchain(0, 1, 512), _xkq(2),
            _xqe(4), _xvq(0, nc.gpsimd), _qtchain(0, 4, 1024),
            _xkq(3), _ktchain(0, 2, 1024), _vsub(0, 0),
            _xqe(5), _qtchain(0, 5, 1280),
            _xqe(6), _xvq(1, nc.gpsimd), _ktchain(0, 3, 1536),
            _qtchain(0, 6, 1536), _vsub(0, 1),
            _xqe(7), _qtchain(0, 7, 1792),
            _xvq(2, nc.gpsimd), _vsub(0, 2),
            _xvq(3, nc.gpsimd), _vsub(0, 3),
        ]
        # w1: kt1 e0-5 + V1 q0,q1; w2: qt1 e0-3 + kt1 e6,e7 + V1 q2,q3;
        # w3: qt1 e4-7 + V2. Fill order in FILLS is kt1, qt1, kt2/qt2,
        # kt3/qt3, so plain _fill/fill_chain pairs walk it. Then
        # w4-w12: block-2/3 fills, V3-V7 interleaved at odd windows.
        def fills(k):
            out = []
            for _ in range(k):
                out += [_fill, fill_chain]
            return out

        WORK[1] = (fills(3) + [_vsub(1, 0)] + fills(3) + [_vsub(1, 1)])
        WORK[2] = ([_wot] + fills(2) + [_vsub(1, 2)] + fills(2) +
                   [_vsub(1, 3)] + fills(2))
        WORK[3] = (fills(2) + [_vsub(2, 0), _vsub(2, 1)] + fills(2) +
                   [_vsub(2, 2), _vsub(2, 3)])
        NFILL = {4: 4, 5: 4, 6: 4, 7: 4, 8: 4, 9: 3, 10: 4, 11: 3, 12: 2}
        VWIN = {5: 3, 7: 4, 9: 5, 11: 6, 13: 7}
        for w in range(4, 15):
            items = []
            nf = NFILL.get(w, 0)
            for j in range(nf):
                items += [_fill, fill_chain]
            if w in VWIN:
                h = VWIN[w]
                vitems = [_vsub(h, qi) for qi in range(4)]
                merged = []
                for a, b in zip(items + [None] * 8, vitems + [None] * 8):
                    if a is not None:
                        merged.append(a)
                    if b is not None:
                        merged.append(b)
                items = merged
            WORK[w] = items
        WORK[15] = []   # FC1 injected once transposes(3, 0) are emitted
        fc1_tail = []   # last FC1 halves run in the tail (PE idles there)

        # ---- steady state: entry list + trailing PV cursor ----
        entries = ([(0, kb, 0) for kb in range(4)] +
                   [(0, kb, 1) for kb in range(4)] +
                   [(0, kb, None) for kb in range(4, NKB)])
        for n in range(1, 16):
            entries += [(n, kb, None) for kb in range(NKB)]
        wstart = {n: (20 if n else 8) + 16 * (n - (0 if n == 0 else 1))
                  for n in range(16)}
        wcount = {n: (12 if n == 0 else 16) for n in range(16)}

        done = {w: 0 for w in range(16)}

        def drain(w, i, sub):
            for pw in range(w):        # flush leftovers of earlier windows
                lst = WORK.get(pw) or []
                while done[pw] < len(lst):
                    lst[done[pw]]()
                    done[pw] += 1
            lst = WORK.get(w)
            if not lst:
                return
            j = i - wstart[w]
            target = min(len(lst), (len(lst) * (2 * j + 1 + sub)
                                    + 2 * wcount[w] - 1) // (2 * wcount[w]))
            while done[w] < target:
                lst[done[w]]()
                done[w] += 1

        def pv_entry(idx, tail=False):
            n, kb, half = entries[idx]
            if vdone[n // 2] * 4 <= kb:   # V chains for this kb not emitted
                return False
            if half is None:
                qbs = range(8)
            else:
                qbs = range(4) if half == 0 else range(4, 8)
            pv_step(n, kb, qbs)
            if kb == NKB - 1 and (half is None or half == 1):
                norms(n)
                if n % 4 == 2:       # qh=0 half of block n//4 complete
                    transposes(n // 4, 0, [nc.sync])
                    if n == 14:
                        halves = [
                            _fc_half(tb, c, (nc.gpsimd, nc.sync)[tb % 2])
                            for tb in range(8) for c in range(2)]
                        WORK[15].extend(halves[:14])
                        fc1_tail.extend(halves[14:])
                elif n % 4 == 3:     # qh=1 half complete
                    engs = [nc.sync, nc.scalar] if tail else [nc.sync]
                    transposes(n // 4, 1, engs)
            return True

        pv_cur = 0
        for i in range(8, len(entries)):
            n, kb, half = entries[i]
            drain(n, i, 0)
            score_tile(n, kb, half)
            while pv_cur <= i - 2 and pv_entry(pv_cur):
                pv_cur += 1
            drain(n, i, 1)
        while pv_cur < len(entries):
            assert pv_entry(pv_cur, tail=True), "V chains missing at tail"
            pv_cur += 1
        for f in fc1_tail:   # fills PE during the blk3/qh1 transpose lead-in
            f()
        for tb in range(8, 16):
            fc_tb(tb, (nc.gpsimd, nc.sync, nc.scalar)[tb % 3])


_CACHED = None


def _build():
    global _CACHED
    if _CACHED is None:
        nc = bacc.Bacc("TRN2", target_bir_lowering=False, debug=False)
        _emit(nc)
        nc.compile()
        _CACHED = nc
    return _CACHED


def _run(inputs, trace=False, trace_kwargs=None):
    """Shard, run on 8 cores, gather. Returns (y, BassKernelResults)."""
    query, key, value = inputs["query"], inputs["key"], inputs["value"]
    Wq, Wk, Wv, Wo = inputs["Wq"], inputs["Wk"], inputs["Wv"], inputs["Wo"]
    bv, bo = inputs["bv"], inputs["bo"]

    f32 = np.float32
    wqT = np.asarray(Wq, f32).T.astype(NPBF16)   # [in, out]
    wkT = np.asarray(Wk, f32).T.astype(NPBF16)
    wvT = np.asarray(Wv, f32).T.astype(NPBF16)
    woT = np.asarray(Wo, f32).T.astype(NPBF16)   # [in(=hd), out]

    xqs = [np.asarray(query[b], f32).T.astype(NPBF16) for b in range(B)]
    xks = [np.asarray(key[b], f32).T.astype(NPBF16) for b in range(B)]
    xvs = [np.asarray(value[b], f32).T.astype(NPBF16) for b in range(B)]

    in_maps = []
    for c in range(NCORES):
        b, hh = divmod(c, 2)
        sl = slice(hh * DHALF, (hh + 1) * DHALF)
        in_maps.append({
            "xq": xqs[b], "xk": xks[b], "xv": xvs[b],
            "wq": np.ascontiguousarray(wqT[:, sl]),
            "wk": np.ascontiguousarray(wkT[:, sl]),
            "wv": np.ascontiguousarray(wvT[:, sl]),
            "wo": np.ascontiguousarray(woT[sl, :]),
        })

    nc = _build()
    kw = {}
    if trace:
        kw["trace"] = True
        kw["trace_kwargs"] = trace_kwargs or {}
    res = run_bass_kernel_spmd(nc, in_maps, core_ids=list(range(NCORES)), **kw)

    # host-side tensor-parallel reduction + exact bias
    bias = (np.asarray(bv, f32) @ np.asarray(Wo, f32).T + np.asarray(bo, f32))
    yout = np.empty((B, T, D), dtype=f32)
    for b in range(B):
        yout[b] = (np.asarray(res.results[2 * b]["y"], f32)
                   + np.asarray(res.results[2 * b + 1]["y"], f32))
        yout[b] += bias[None, :]
    return yout, res


def kernel(**inputs):
    yv, _ = _run(inputs, trace=False)
    return yv


# revision 46
# speedup vs baseline: 1.0094x; 1.0094x over previous
"""Multi-head attention (B=4, T=2048, D=1024, H=16) on 8 TRN2 NeuronCores.

Sharding: batch x head-half (4 batches x 2 halves of 8 heads = 8 cores).
Each core projects Q/K/V for its 8 heads over the full 2048 tokens, runs
attention, and computes partial output projections against its half of Wo.
The tensor-parallel FC "all-reduce" is a host-side sum of the partials.

Per-core program (all matmul inputs bf16, fp32 PSUM accumulation):
  - Scores S = K_blk @ Q^T land as [128 ktok, 1024 q] PSUM tiles; one exp
    per tile (scalar engine) writes P directly as bf16. The exp stream is
    the pacer (~266us); emission follows a global entry cursor: the score
    matmuls for entry i are gated on exp(i-2) freeing a PSUM slot, and the
    PV steps trail two entries behind on the same gates.
  - PV is kb-major and output-stationary: 8 PSUM accumulators [128 q, 65]
    (ones-augmented V gives the softmax denominator in col 64) accumulate
    one k-block right after its exp, so PV(n) completes with exp(n,15).
  - Normalization (DVE reciprocal + scalar multiply) writes two heads of a
    block side-by-side into [128 q, 128 hd] pair tiles; a DMA-engine XBAR
    transpose moves them into head-major otT with zero PE cost.
  - The FC accumulates all 4 head-pair blocks in PSUM per token block; the
    qh=0 half runs inside the last exp window, the qh=1 half trails PV(15).
  - Bootstrap: wq rides the gpsimd queue and the first xq eighths ride the
    small c-slots so the first exp fires ~9us in; the first four score
    tiles are split into 512-wide halves to keep the exp cursor moving
    while projections catch up. xk quarters hand their slots straight to
    xv (block-1 K/Q re-load as eighths in w1/w2, like the block-2/3 fills).

Host side: transposes inputs to feature-major bf16, slices weights per
head-half, runs SPMD on 8 cores, sums the two partial y per batch, and
adds the exact (bv @ Wo.T + bo) bias (attention rows sum to 1 so the value
bias passes through; bq/bk are zero in this problem).
"""
import numpy as np
from contextlib import ExitStack

import ml_dtypes

import concourse.bass as bass
import concourse.tile as tile
from concourse import bacc, mybir
from concourse.bass_utils import run_bass_kernel_spmd

F32 = mybir.dt.float32
BF16 = mybir.dt.bfloat16
NPBF16 = ml_dtypes.bfloat16

B = 4
T = 2048
D = 1024
H = 16
DK = 64
NCORES = 8
HLOC = 8           # heads per core
DHALF = 512        # hd dims per core
NKB = T // 128     # 16 key blocks
EXP_SCALE = 1.0 / np.sqrt(DK)


def _emit(nc):
    xq = nc.dram_tensor("xq", [D, T], BF16, kind="ExternalInput").ap()   # query^T
    xk = nc.dram_tensor("xk", [D, T], BF16, kind="ExternalInput").ap()   # key^T
    xv = nc.dram_tensor("xv", [D, T], BF16, kind="ExternalInput").ap()   # value^T
    wq = nc.dram_tensor("wq", [D, DHALF], BF16, kind="ExternalInput").ap()
    wk = nc.dram_tensor("wk", [D, DHALF], BF16, kind="ExternalInput").ap()
    wv = nc.dram_tensor("wv", [D, DHALF], BF16, kind="ExternalInput").ap()
    wo = nc.dram_tensor("wo", [DHALF, D], BF16, kind="ExternalInput").ap()
    y = nc.dram_tensor("y", [T, D], BF16, kind="ExternalOutput").ap()  # partial

    with tile.TileContext(nc) as tc, ExitStack() as ctx:
        res = ctx.enter_context(tc.tile_pool(name="res", bufs=1))
        otT = res.tile([128, 4, T], BF16)      # normalized attention out^T
        vaug = res.tile([128, NKB, HLOC, DK + 1], BF16)
        wot = res.tile([128, 4, D], BF16)      # Wo^T slice [(ki p) m -> p ki m]
        zz = res.tile([1, 512], BF16)          # zero row for PSUM-bank zeroing
        nc.vector.memset(vaug[:, :, :, DK:DK + 1], 1.0)
        nc.vector.memset(zz[:], 0.0)

        # K^T / Q^T blocks [128 hd, 2048 tok], two rotating slots per tag.
        ktq = ctx.enter_context(tc.tile_pool(name="ktq", bufs=2))
        kts, qts = {}, {}

        wst = ctx.enter_context(tc.tile_pool(name="wst", bufs=1))
        wk_s = wst.tile([128, 8, DHALF], BF16, name="wk_s", tag="wk")
        wq_s = wst.tile([128, 8, DHALF], BF16, name="wq_s", tag="wq")
        wv_s = wst.tile([128, 8, DHALF], BF16, name="wv_s", tag="wv")

        # Input staging: four quarter slots [128, 8, 512] carry xk then xv
        # (xv pinned for the per-head V quanta); slot c is a 2-deep eighth
        # slab [128, 8, 256] for all xq traffic and the K/Q re-load fills.
        xst = ctx.enter_context(tc.tile_pool(name="xst", bufs=1))

        # PSUM (8 banks): score/FC tiles [128,1024] x2 (4 banks), PV
        # accumulators [128,4,128] x2 (2), projection chains [128,512] x2.
        bigp = ctx.enter_context(tc.tile_pool(name="bigp", bufs=2, space="PSUM"))
        pvp = ctx.enter_context(tc.tile_pool(name="pvp", bufs=1, space="PSUM"))
        pva = pvp.tile([128, 4, 128], F32, name="pva", tag="pva")
        pvb = pvp.tile([128, 4, 128], F32, name="pvb", tag="pvb")
        smp = ctx.enter_context(tc.tile_pool(name="smp", bufs=2, space="PSUM"))

        ptp = ctx.enter_context(tc.tile_pool(name="ptp", bufs=14))  # P ring
        nrm = ctx.enter_context(tc.tile_pool(name="nrm", bufs=1))   # recip
        prp = ctx.enter_context(tc.tile_pool(name="prp", bufs=2))   # pair tiles
        evp = ctx.enter_context(tc.tile_pool(name="evp", bufs=1))   # fc evict
        fpp = ctx.enter_context(tc.tile_pool(name="fpp", bufs=1))   # fc2 partials

        def load_quarter(src, i, slot, eng):
            xs = xst.tile([128, 8, 512], BF16, name=f"x_{slot}", tag=slot)
            eng.dma_start(
                xs, src[:, i * 512:(i + 1) * 512]
                .rearrange("(ki p) t -> p ki t", p=128))
            return xs

        def load_eighth(src, e, eng):
            xs = xst.tile([128, 8, 256], BF16, name="x_c", tag="c", bufs=2)
            eng.dma_start(
                xs, src[:, e * 256:(e + 1) * 256]
                .rearrange("(ki p) t -> p ki t", p=128))
            return xs

        written = {}   # (which, blk) -> set of written 128-col chunks

        def kq_chain(w_s, blk, xs, xcol, width, dst, dstcol):
            """8-ki projection chain xs[:,:,xcol:xcol+width] -> dst cols."""
            which = "k" if w_s is wk_s else "q"
            written.setdefault((which, blk), set()).update(
                range(dstcol // 128, (dstcol + width) // 128))
            ps = smp.tile([128, 512], F32, name="pps", tag="sm")
            for ki in range(8):
                nc.tensor.matmul(
                    ps[:, 0:width],
                    lhsT=w_s[:, ki, blk * 128:(blk + 1) * 128],
                    rhs=xs[:, ki, xcol:xcol + width],
                    start=(ki == 0), stop=(ki == 7))
            nc.vector.tensor_copy(dst[:, dstcol:dstcol + width], ps[:, 0:width])

        def blk_tile(which, blk):
            tiles = kts if which == "k" else qts
            if blk not in tiles:
                tiles[blk] = ktq.tile([128, T], BF16,
                                      name=f"{which}t{blk}",
                                      tag=which)
            return tiles[blk]

        # K/Q re-load fills via the 2-deep c slot (blocks 1-3).
        fill_q = []

        def fill_dma(blk, which, e):
            src = xk if which == "k" else xq
            xs = load_eighth(src, e, nc.sync)
            fill_q.append((blk, which, e, xs))

        def fill_chain():
            blk, which, e, xs = fill_q.pop(0)
            w_s = wk_s if which == "k" else wq_s
            kq_chain(w_s, blk, xs, 0, 256, blk_tile(which, blk), e * 256)

        xv_q = [None] * 4

        def v_subq(h, qi):
            """V projection for head h, token blocks 4*qi..4*qi+4."""
            xs = xv_q[qi]
            for tb in range(4 * qi, 4 * qi + 4):
                ps = smp.tile([128, 512], F32, name="vps", tag="sm")
                for ki in range(8):
                    nc.tensor.matmul(
                        ps[:, 0:DK],
                        lhsT=xs[:, ki, (tb % 4) * 128:(tb % 4 + 1) * 128],
                        rhs=wv_s[:, ki, h * DK:(h + 1) * DK],
                        start=(ki == 0), stop=(ki == 7))
                nc.vector.tensor_copy(vaug[:, tb, h, 0:DK], ps[:, 0:DK])

        pts = {}

        def score_tile(n, kb, half=None):
            """Score matmuls + exp for tile (n, kb); half 0/1 = 512-wide."""
            h, qh = divmod(n, 2)
            blk, po = h // 2, (h % 2) * 64
            ktb, qtb = kts[blk], qts[blk]
            if (n, kb) not in pts:
                pts[(n, kb)] = ptp.tile([128, 1024], BF16,
                                        name="pt", tag="pt")
            pt = pts[(n, kb)]
            cs = (0, 1) if half is None else (half,)
            assert kb in written[("k", blk)], f"S({n},{kb}): kt{blk} race"
            for c in cs:
                need_q = range(8 * qh + 4 * c, 8 * qh + 4 * c + 4)
                assert written[("q", blk)].issuperset(need_q), \
                    f"S({n},{kb},{c}): qt{blk} race"
            st = bigp.tile([128, 1024], F32, name="st", tag="big")
            for c in cs:
                nc.tensor.matmul(
                    st[:, c * 512:(c + 1) * 512],
                    lhsT=ktb[po:po + 64, kb * 128:(kb + 1) * 128],
                    rhs=qtb[po:po + 64,
                            qh * 1024 + c * 512:qh * 1024 + (c + 1) * 512],
                    start=True, stop=True)
            c0, width = cs[0] * 512, len(cs) * 512
            nc.scalar.activation(
                pt[:, c0:c0 + width], st[:, c0:c0 + width],
                mybir.ActivationFunctionType.Exp, scale=EXP_SCALE)

        def pv_zero(accs=(0, 1)):
            """Zero PV accumulator banks with one start=True matmul each.
            PSUM zero-region granularity is the whole 2KB bank, so per-slot
            start=True flags from the interleaved qb series would re-mark
            each other's bytes and drop contributions; an explicit full-bank
            start + accumulate-only steps is the safe pattern."""
            for a in accs:
                acc = (pva, pvb)[a]
                nc.tensor.matmul(
                    acc.rearrange("p a b -> p (a b)"),
                    lhsT=zz[:, 0:128], rhs=zz[:, 0:512],
                    start=True, stop=False)

        def pv_step(n, kb, qbs):
            """PV accumulation of k-block kb for window n, given q-blocks."""
            h = n // 2
            if kb == 0 and n > 0 and qbs[0] == 0:
                pv_zero((0, 1))   # re-zero banks; norms(n-1) reads are done
            if qbs[-1] == 7:      # last reader of this pt tile
                pt = pts.pop((n, kb))
            else:
                pt = pts[(n, kb)]
            for qb in qbs:
                acc = (pva, pvb)[qb // 4]
                nc.tensor.matmul(
                    acc[:, qb % 4, 0:DK + 1],
                    lhsT=pt[:, qb * 128:(qb + 1) * 128],
                    rhs=vaug[:, kb, h, :],
                    start=False, stop=(kb == NKB - 1))

        pairs = {}

        def norms(n):
            """Normalize window n into pair tiles [128 q, 128 hd]. One
            batched reciprocal per bank (4 denominators), then 4 scalar
            multiplies; each bank is re-zeroed for the next window as soon
            as its four reads are done so the PE stall at the window
            boundary is short."""
            h, qh = divmod(n, 2)
            co = (h % 2) * 64
            for a, acc in enumerate((pva, pvb)):
                rd = nrm.tile([128, 4], F32, name="rd", tag="rd", bufs=2)
                nc.vector.reciprocal(rd[:], acc[:, :, DK:DK + 1])
                for s in range(4):
                    qb = 4 * a + s
                    if (qh, qb) not in pairs:
                        pairs[(qh, qb)] = prp.tile(
                            [128, 128], BF16, name=f"pr{qh}_{qb}",
                            tag=f"pr{qh}{qb}")
                    nc.vector.tensor_scalar_mul(
                        pairs[(qh, qb)][:, co:co + DK], acc[:, s, 0:DK],
                        rd[:, s:s + 1])

        tdone = set()

        def transposes(blk, qh, engs):
            """XBAR DMA transpose pair tiles -> otT for (blk, qh)."""
            tdone.add((blk, qh))
            for qb in range(8):
                pr = pairs.pop((qh, qb))
                engs[qb % len(engs)].dma_start_transpose(
                    otT[:, blk, qh * 1024 + qb * 128:qh * 1024 + (qb + 1) * 128],
                    pr)

        def fc_tb(tb, eng, use_smp=False):
            """Output projection for one token block (all 4 ki accumulated).
            Evictions rotate through dead staging slots. use_smp runs the
            two 512-col halves through the small PSUM pool so the score
            pipeline keeps both bigp slots during the last exp window."""
            ev_slots = ["s0", "s1", "s2", "s3", "wk", "wq", "c", "ev"]
            slot = ev_slots[tb % 8]
            pool = {"wk": wst, "wq": wst, "ev": evp}.get(slot, xst)
            ev = pool.tile([128, 1024], BF16, name="ev", tag=slot,
                           bufs=2 if slot == "c" else 1)
            if use_smp:
                for c in range(2):
                    fp = smp.tile([128, 512], F32, name="fch", tag="sm")
                    for ki in range(4):
                        nc.tensor.matmul(
                            fp[:],
                            lhsT=otT[:, ki, tb * 128:(tb + 1) * 128],
                            rhs=wot[:, ki, c * 512:(c + 1) * 512],
                            start=(ki == 0), stop=(ki == 3))
                    nc.vector.tensor_copy(ev[:, c * 512:(c + 1) * 512], fp[:])
            else:
                fp = bigp.tile([128, 1024], F32, name="fcp", tag="big")
                for c in range(2):
                    for ki in range(4):
                        nc.tensor.matmul(
                            fp[:, c * 512:(c + 1) * 512],
                            lhsT=otT[:, ki, tb * 128:(tb + 1) * 128],
                            rhs=wot[:, ki, c * 512:(c + 1) * 512],
                            start=(ki == 0), stop=(ki == 3))
                    nc.vector.tensor_copy(
                        ev[:, c * 512:(c + 1) * 512],
                        fp[:, c * 512:(c + 1) * 512])
                    eng.dma_start(
                        y[tb * 128:(tb + 1) * 128, c * 512:(c + 1) * 512],
                        ev[:, c * 512:(c + 1) * 512])
                return
            eng.dma_start(y[tb * 128:(tb + 1) * 128, :], ev[:])

        # ---- bootstrap ----
        # Queues: sync=SP, scalar=ACT, gpsimd=Pool(SWDGE). wq rides gpsimd,
        # wk + the xq eighths ride scalar, xk/xv quarters ride sync/gpsimd.
        nc.gpsimd.dma_start(wq_s[:, :, 0:256],
                            wq[:, 0:256].rearrange("(ki p) m -> p ki m", p=128))
        xqe = [load_eighth(xq, e, nc.scalar) for e in range(2)]
        nc.scalar.dma_start(wk_s[:, :, 0:256],
                            wk[:, 0:256].rearrange("(ki p) m -> p ki m", p=128))
        xk_q = [load_quarter(xk, 0, "s0", nc.sync)]

        # PE p-state warmup: the cost model runs the PE at 1.2 GHz until it
        # has been continuously busy for 3us. Junk matmuls bridge the DMA
        # wait so the real chains start at full speed (2.4 GHz).
        pv_zero()
        for j in range(5):
            jt = bigp.tile([128, 1024], F32, name="jnk", tag="big")
            nc.tensor.matmul(jt[:, 0:512], lhsT=zz[:, 0:128],
                             rhs=zz[:, 0:512], start=True, stop=True)

        blk_tile("k", 0)
        blk_tile("q", 0)

        # First chains: qt0 tokens 0:512 from the c-slot eighths (their DMAs
        # land first), then kt0 tokens 0:512 in 128-wide slices.
        kq_chain(wq_s, 0, xqe[0], 0, 256, qts[0], 0)
        kq_chain(wq_s, 0, xqe[1], 0, 256, qts[0], 256)
        for s in range(4):
            kq_chain(wk_s, 0, xk_q[0], s * 128, 128, kts[0], s * 128)
        nc.gpsimd.dma_start(wq_s[:, :, 256:512],
                            wq[:, 256:512].rearrange("(ki p) m -> p ki m", p=128))
        xqe += [load_eighth(xq, e, nc.scalar) for e in range(2, 4)]
        nc.scalar.dma_start(wk_s[:, :, 256:512],
                            wk[:, 256:512].rearrange("(ki p) m -> p ki m", p=128))
        nc.gpsimd.dma_start(wv_s, wv.rearrange("(ki p) m -> p ki m", p=128))
        xk_q.append(load_quarter(xk, 1, "s1", nc.sync))

        # First four score tiles in 512-wide halves to start the exp stream
        # while qt0's second half is still projecting.
        for kb in range(4):
            score_tile(0, kb, half=0)
        kq_chain(wq_s, 0, xqe[2], 0, 256, qts[0], 512)
        kq_chain(wq_s, 0, xqe[3], 0, 256, qts[0], 768)
        for kb in range(4):
            score_tile(0, kb, half=1)

        # ---- per-window work queues ----
        WORK = {w: [] for w in range(16)}

        def _xkq(i):
            def f():
                xk_q.append(load_quarter(xk, i, f"s{i}", nc.sync))
            return f

        def _ktchain(blk, i, dstcol):
            return lambda: kq_chain(wk_s, blk, xk_q[i], 0, 512,
                                    blk_tile("k", blk), dstcol)

        def _xqe(e):
            def f():
                xqe.append(load_eighth(xq, e, nc.scalar))
            return f

        def _qtchain(blk, e, dstcol):
            return lambda: kq_chain(wq_s, blk, xqe[e], 0, 256,
                                    blk_tile("q", blk), dstcol)

        def _xvq(i, eng):
            def f():
                xv_q[i] = load_quarter(xv, i, f"s{i}", eng)
            return f

        vdone = [0] * 8

        def _vsub(h, qi):
            def f():
                v_subq(h, qi)
                vdone[h] += 1
            return f

        def _wot():
            nc.gpsimd.dma_start(wot, wo.rearrange("(ki p) m -> p ki m", p=128))

        FILLS = ([(1, "k", e) for e in range(8)] +
                 [(1, "q", e) for e in range(8)] +
                 [(2, "k", e) for e in range(8)] +
                 [(2, "q", e) for e in range(8)] +
                 [(3, "k", e) for e in range(8)] +
                 [(3, "q", e) for e in range(8)])
        fill_i = [0]

        def _fill():
            blk, which, e = FILLS[fill_i[0]]
            fill_i[0] += 1
            fill_dma(blk, which, e)

        def _fc_half(tb, c, eng):
            """One 512-col half of FC for tb through the small PSUM pool;
            eviction + store on the c=1 half."""
            def f():
                ev_slots = ["s0", "s1", "s2", "s3", "wk", "wq", "c", "ev"]
                slot = ev_slots[tb % 8]
                pool = {"wk": wst, "wq": wst, "ev": evp}.get(slot, xst)
                if tb not in fc_ev:
                    fc_ev[tb] = pool.tile([128, 1024], BF16, name="ev",
                                          tag=slot,
                                          bufs=2 if slot == "c" else 1)
                ev = fc_ev[tb]
                fp = smp.tile([128, 512], F32, name="fch", tag="sm")
                for ki in range(4):
                    nc.tensor.matmul(
                        fp[:],
                        lhsT=otT[:, ki, tb * 128:(tb + 1) * 128],
                        rhs=wot[:, ki, c * 512:(c + 1) * 512],
                        start=(ki == 0), stop=(ki == 3))
                nc.vector.tensor_copy(ev[:, c * 512:(c + 1) * 512], fp[:])
                if c == 1:
                    eng.dma_start(y[tb * 128:(tb + 1) * 128, :], ev[:])
            return f

        fc_ev = {}
        fcp = {}

        def _fc2_part(tb):
            """ki0-2 partial of FC for token block tb (qh=1), to SBUF bf16.
            Runs during windows 12-14 so only the blk3 contraction and an
            add remain after the last exp."""
            def f():
                assert {(0, 1), (1, 1), (2, 1)} <= tdone, f"fc2 part {tb} race"
                fcp[tb] = fpp.tile([128, 1024], BF16, name=f"fcp{tb}",
                                   tag=f"fcp{tb}")
                for c in range(2):
                    fp = smp.tile([128, 512], F32, name="fc2p", tag="sm")
                    for ki in range(3):
                        nc.tensor.matmul(
                            fp[:],
                            lhsT=otT[:, ki, tb * 128:(tb + 1) * 128],
                            rhs=wot[:, ki, c * 512:(c + 1) * 512],
                            start=(ki == 0), stop=(ki == 2))
                    nc.vector.tensor_copy(
                        fcp[tb][:, c * 512:(c + 1) * 512], fp[:])
            return f

        def _fc2_fin(tb, eng):
            """Tail: blk3 contraction + add of the ki0-2 partial + store."""
            assert (3, 1) in tdone, f"fc2 fin {tb} race"
            ev_slots = ["s0", "s1", "s2", "s3", "wk", "wq", "c", "ev"]
            slot = ev_slots[tb % 8]
            pool = {"wk": wst, "wq": wst, "ev": evp}.get(slot, xst)
            ev = pool.tile([128, 1024], BF16, name="ev", tag=slot,
                           bufs=2 if slot == "c" else 1)
            for c in range(2):
                fp = smp.tile([128, 512], F32, name="fc2f", tag="sm")
                nc.tensor.matmul(
                    fp[:],
                    lhsT=otT[:, 3, tb * 128:(tb + 1) * 128],
                    rhs=wot[:, 3, c * 512:(c + 1) * 512],
                    start=True, stop=True)
                nc.vector.tensor_add(
                    ev[:, c * 512:(c + 1) * 512], fp[:],
                    fcp[tb][:, c * 512:(c + 1) * 512])
                eng.dma_start(
                    y[tb * 128:(tb + 1) * 128, c * 512:(c + 1) * 512],
                    ev[:, c * 512:(c + 1) * 512])

        # w0: rest of kt0 (tokens 512:2048 from xk quarters as they land),
        # qt0 tokens 1024:2048 (xq eighths 4-7), xv quarter loads, V0.
        # Order matters: the kt0 chain feeding score tiles (0, 4e..4e+4)
        # must be EMITTED before those score tiles (drain index < 4 * e + 4),
        # else the tile framework sees the read first (race -> garbage).
        WORK[0] = [
            _ktchain(0, 1, 512), _xkq(2),
            _xqe(4), _xvq(0, nc.gpsimd), _qtchain(0, 4, 1024),
            _xkq(3), _ktchain(0, 2, 1024), _vsub(0, 0),
            _xqe(5), _qtchain(0, 5, 1280),
            _xqe(6), _xvq(1, nc.gpsimd), _ktchain(0, 3, 1536),
            _qtchain(0, 6, 1536), _vsub(0, 1),
            _xqe(7), _qtchain(0, 7, 1792),
            _xvq(2, nc.gpsimd), _vsub(0, 2),
            _xvq(3, nc.gpsimd), _vsub(0, 3),
        ]
        # w1: kt1 e0-5 + V1 q0,q1; w2: qt1 e0-3 + kt1 e6,e7 + V1 q2,q3;
        # w3: qt1 e4-7 + V2. Fill order in FILLS is kt1, qt1, kt2/qt2,
        # kt3/qt3, so plain _fill/fill_chain pairs walk it. Then
        # w4-w12: block-2/3 fills, V3-V7 interleaved at odd windows.
        def fills(k):
            out = []
            for _ in range(k):
                out += [_fill, fill_chain]
            return out

        WORK[1] = (fills(3) + [_vsub(1, 0)] + fills(3) + [_vsub(1, 1)])
        WORK[2] = ([_wot] + fills(2) + [_vsub(1, 2)] + fills(2) +
                   [_vsub(1, 3)] + fills(2))
        WORK[3] = (fills(2) + [_vsub(2, 0), _vsub(2, 1)] + fills(2) +
                   [_vsub(2, 2), _vsub(2, 3)])
        NFILL = {4: 4, 5: 4, 6: 4, 7: 4, 8: 4, 9: 3, 10: 4, 11: 3, 12: 2}
        FC2P = {12: (8, 9), 13: (10, 11), 14: (12, 13, 14, 15)}
        VWIN = {5: 3, 7: 4, 9: 5, 11: 6, 13: 7}
        for w in range(4, 15):
            items = []
            nf = NFILL.get(w, 0)
            for j in range(nf):
                items += [_fill, fill_chain]
            if w in VWIN:
                h = VWIN[w]
                vitems = [_vsub(h, qi) for qi in range(4)]
                merged = []
                for a, b in zip(items + [None] * 8, vitems + [None] * 8):
                    if a is not None:
                        merged.append(a)
                    if b is not None:
                        merged.append(b)
                items = merged
            items += [_fc2_part(tb) for tb in FC2P.get(w, ())]
            WORK[w] = items
        WORK[15] = []   # FC1 injected once transposes(3, 0) are emitted
        fc1_tail = []   # last FC1 halves run in the tail (PE idles there)

        # ---- steady state: entry list + trailing PV cursor ----
        entries = ([(0, kb, 0) for kb in range(4)] +
                   [(0, kb, 1) for kb in range(4)] +
                   [(0, kb, None) for kb in range(4, NKB)])
        for n in range(1, 16):
            entries += [(n, kb, None) for kb in range(NKB)]
        wstart = {n: (20 if n else 8) + 16 * (n - (0 if n == 0 else 1))
                  for n in range(16)}
        wcount = {n: (12 if n == 0 else 16) for n in range(16)}

        done = {w: 0 for w in range(16)}

        def drain(w, i, sub):
            for pw in range(w):        # flush leftovers of earlier windows
                lst = WORK.get(pw) or []
                while done[pw] < len(lst):
                    lst[done[pw]]()
                    done[pw] += 1
            lst = WORK.get(w)
            if not lst:
                return
            j = i - wstart[w]
            target = min(len(lst), (len(lst) * (2 * j + 1 + sub)
                                    + 2 * wcount[w] - 1) // (2 * wcount[w]))
            while done[w] < target:
                lst[done[w]]()
                done[w] += 1

        def pv_entry(idx, tail=False):
            n, kb, half = entries[idx]
            if vdone[n // 2] * 4 <= kb:   # V chains for this kb not emitted
                return False
            if half is None:
                qbs = range(8)
            else:
                qbs = range(4) if half == 0 else range(4, 8)
            pv_step(n, kb, qbs)
            if kb == NKB - 1 and (half is None or half == 1):
                norms(n)
                if n % 4 == 2:       # qh=0 half of block n//4 complete
                    transposes(n // 4, 0, [nc.sync])
                    if n == 14:
                        halves = [
                            _fc_half(tb, c, (nc.gpsimd, nc.sync)[tb % 2])
                            for tb in range(8) for c in range(2)]
                        WORK[15].extend(halves[:14])
                        fc1_tail.extend(halves[14:])
                elif n % 4 == 3:     # qh=1 half complete
                    engs = [nc.sync, nc.scalar] if tail else [nc.sync]
                    transposes(n // 4, 1, engs)
            return True

        pv_cur = 0
        for i in range(8, len(entries)):
            n, kb, half = entries[i]
            drain(n, i, 0)
            score_tile(n, kb, half)
            while pv_cur <= i - 2 and pv_entry(pv_cur):
                pv_cur += 1
            drain(n, i, 1)
        while pv_cur < len(entries):
            assert pv_entry(pv_cur, tail=True), "V chains missing at tail"
            pv_cur += 1
        for f in fc1_tail:   # fills PE during the blk3/qh1 transpose lead-in
            f()
        for tb in range(8, 16):
            _fc2_fin(tb, (nc.gpsimd, nc.sync, nc.scalar)[tb % 3])


_CACHED = None


def _build():
    global _CACHED
    if _CACHED is None:
        nc = bacc.Bacc("TRN2", target_bir_lowering=False, debug=False)
        _emit(nc)
        nc.compile()
        _CACHED = nc
    return _CACHED


def _run(inputs, trace=False, trace_kwargs=None):
    """Shard, run on 8 cores, gather. Returns (y, BassKernelResults)."""
    query, key, value = inputs["query"], inputs["key"], inputs["value"]
    Wq, Wk, Wv, Wo = inputs["Wq"], inputs["Wk"], inputs["Wv"], inputs["Wo"]
    bv, bo = inputs["bv"], inputs["bo"]

    f32 = np.float32
    wqT = np.asarray(Wq, f32).T.astype(NPBF16)   # [in, out]
    wkT = np.asarray(Wk, f32).T.astype(NPBF16)
    wvT = np.asarray(Wv, f32).T.astype(NPBF16)
    woT = np.asarray(Wo, f32).T.astype(NPBF16)   # [in(=hd), out]

    xqs = [np.asarray(query[b], f32).T.astype(NPBF16) for b in range(B)]
    xks = [np.asarray(key[b], f32).T.astype(NPBF16) for b in range(B)]
    xvs = [np.asarray(value[b], f32).T.astype(NPBF16) for b in range(B)]

    in_maps = []
    for c in range(NCORES):
        b, hh = divmod(c, 2)
        sl = slice(hh * DHALF, (hh + 1) * DHALF)
        in_maps.append({
            "xq": xqs[b], "xk": xks[b], "xv": xvs[b],
            "wq": np.ascontiguousarray(wqT[:, sl]),
            "wk": np.ascontiguousarray(wkT[:, sl]),
            "wv": np.ascontiguousarray(wvT[:, sl]),
            "wo": np.ascontiguousarray(woT[sl, :]),
        })

    nc = _build()
    kw = {}
    if trace:
        kw["trace"] = True
        kw["trace_kwargs"] = trace_kwargs or {}
    res = run_bass_kernel_spmd(nc, in_maps, core_ids=list(range(NCORES)), **kw)

    # host-side tensor-parallel reduction + exact bias
    bias = (np.asarray(bv, f32) @ np.asarray(Wo, f32).T + np.asarray(bo, f32))
    yout = np.empty((B, T, D), dtype=f32)
    for b in range(B):
        yout[b] = (np.asarray(res.results[2 * b]["y"], f32)
                   + np.asarray(res.results[2 * b + 1]["y"], f32))
        yout[b] += bias[None, :]
    return yout, res


def kernel(**inputs):
    yv, _ = _run(inputs, trace=False)
    return yv


# revision 49
# speedup vs baseline: 1.0101x; 1.0007x over previous
"""Multi-head attention (B=4, T=2048, D=1024, H=16) on 8 TRN2 NeuronCores.

Sharding: batch x head-half (4 batches x 2 halves of 8 heads = 8 cores).
Each core projects Q/K/V for its 8 heads over the full 2048 tokens, runs
attention, and computes partial output projections against its half of Wo.
The tensor-parallel FC "all-reduce" is a host-side sum of the partials.

Per-core program (all matmul inputs bf16, fp32 PSUM accumulation):
  - Scores S = K_blk @ Q^T land as [128 ktok, 1024 q] PSUM tiles; one exp
    per tile (scalar engine) writes P directly as bf16. The exp stream is
    the pacer (~266us); emission follows a global entry cursor: the score
    matmuls for entry i are gated on exp(i-2) freeing a PSUM slot, and the
    PV steps trail two entries behind on the same gates.
  - PV is kb-major and output-stationary: 8 PSUM accumulators [128 q, 65]
    (ones-augmented V gives the softmax denominator in col 64) accumulate
    one k-block right after its exp, so PV(n) completes with exp(n,15).
  - Normalization (DVE reciprocal + scalar multiply) writes two heads of a
    block side-by-side into [128 q, 128 hd] pair tiles; a DMA-engine XBAR
    transpose moves them into head-major otT with zero PE cost.
  - The FC accumulates all 4 head-pair blocks in PSUM per token block; the
    qh=0 half runs inside the last exp window, the qh=1 half trails PV(15).
  - Bootstrap: wq rides the gpsimd queue and the first xq eighths ride the
    small c-slots so the first exp fires ~9us in; the first four score
    tiles are split into 512-wide halves to keep the exp cursor moving
    while projections catch up. xk quarters hand their slots straight to
    xv (block-1 K/Q re-load as eighths in w1/w2, like the block-2/3 fills).

Host side: transposes inputs to feature-major bf16, slices weights per
head-half, runs SPMD on 8 cores, sums the two partial y per batch, and
adds the exact (bv @ Wo.T + bo) bias (attention rows sum to 1 so the value
bias passes through; bq/bk are zero in this problem).
"""
import numpy as np
from contextlib import ExitStack

import ml_dtypes

import concourse.bass as bass
import concourse.tile as tile
from concourse import bacc, mybir
from concourse.bass_utils import run_bass_kernel_spmd

F32 = mybir.dt.float32
BF16 = mybir.dt.bfloat16
NPBF16 = ml_dtypes.bfloat16

B = 4
T = 2048
D = 1024
H = 16
DK = 64
NCORES = 8
HLOC = 8           # heads per core
DHALF = 512        # hd dims per core
NKB = T // 128     # 16 key blocks
EXP_SCALE = 1.0 / np.sqrt(DK)


def _emit(nc):
    xq = nc.dram_tensor("xq", [D, T], BF16, kind="ExternalInput").ap()   # query^T
    xk = nc.dram_tensor("xk", [D, T], BF16, kind="ExternalInput").ap()   # key^T
    xv = nc.dram_tensor("xv", [D, T], BF16, kind="ExternalInput").ap()   # value^T
    wq = nc.dram_tensor("wq", [D, DHALF], BF16, kind="ExternalInput").ap()
    wk = nc.dram_tensor("wk", [D, DHALF], BF16, kind="ExternalInput").ap()
    wv = nc.dram_tensor("wv", [D, DHALF], BF16, kind="ExternalInput").ap()
    wo = nc.dram_tensor("wo", [DHALF, D], BF16, kind="ExternalInput").ap()
    y = nc.dram_tensor("y", [T, D], BF16, kind="ExternalOutput").ap()  # partial

    with tile.TileContext(nc) as tc, ExitStack() as ctx:
        res = ctx.enter_context(tc.tile_pool(name="res", bufs=1))
        otT = res.tile([128, 4, T], BF16)      # normalized attention out^T
        vaug = res.tile([128, NKB, HLOC, DK + 1], BF16)
        wot = res.tile([128, 4, D], BF16)      # Wo^T slice [(ki p) m -> p ki m]
        zz = res.tile([1, 512], BF16)          # zero row for PSUM-bank zeroing
        nc.vector.memset(vaug[:, :, :, DK:DK + 1], 1.0)
        nc.vector.memset(zz[:], 0.0)

        # K^T / Q^T blocks [128 hd, 2048 tok], two rotating slots per tag.
        ktq = ctx.enter_context(tc.tile_pool(name="ktq", bufs=2))
        kts, qts = {}, {}

        wst = ctx.enter_context(tc.tile_pool(name="wst", bufs=1))
        wk_s = wst.tile([128, 8, DHALF], BF16, name="wk_s", tag="wk")
        wq_s = wst.tile([128, 8, DHALF], BF16, name="wq_s", tag="wq")
        wv_s = wst.tile([128, 8, DHALF], BF16, name="wv_s", tag="wv")

        # Input staging: four quarter slots [128, 8, 512] carry xk then xv
        # (xv pinned for the per-head V quanta); slot c is a 2-deep eighth
        # slab [128, 8, 256] for all xq traffic and the K/Q re-load fills.
        xst = ctx.enter_context(tc.tile_pool(name="xst", bufs=1))

        # PSUM (8 banks): score/FC tiles [128,1024] x2 (4 banks), PV
        # accumulators [128,4,128] x2 (2), projection chains [128,512] x2.
        bigp = ctx.enter_context(tc.tile_pool(name="bigp", bufs=2, space="PSUM"))
        pvp = ctx.enter_context(tc.tile_pool(name="pvp", bufs=1, space="PSUM"))
        pva = pvp.tile([128, 4, 128], F32, name="pva", tag="pva")
        pvb = pvp.tile([128, 4, 128], F32, name="pvb", tag="pvb")
        smp = ctx.enter_context(tc.tile_pool(name="smp", bufs=2, space="PSUM"))

        ptp = ctx.enter_context(tc.tile_pool(name="ptp", bufs=14))  # P ring
        nrm = ctx.enter_context(tc.tile_pool(name="nrm", bufs=1))   # recip
        prp = ctx.enter_context(tc.tile_pool(name="prp", bufs=2))   # pair tiles
        evp = ctx.enter_context(tc.tile_pool(name="evp", bufs=1))   # fc evict
        fpp = ctx.enter_context(tc.tile_pool(name="fpp", bufs=1))   # fc2 partials

        def load_quarter(src, i, slot, eng):
            xs = xst.tile([128, 8, 512], BF16, name=f"x_{slot}", tag=slot)
            eng.dma_start(
                xs, src[:, i * 512:(i + 1) * 512]
                .rearrange("(ki p) t -> p ki t", p=128))
            return xs

        def load_eighth(src, e, eng):
            xs = xst.tile([128, 8, 256], BF16, name="x_c", tag="c", bufs=2)
            eng.dma_start(
                xs, src[:, e * 256:(e + 1) * 256]
                .rearrange("(ki p) t -> p ki t", p=128))
            return xs

        written = {}   # (which, blk) -> set of written 128-col chunks

        def kq_chain(w_s, blk, xs, xcol, width, dst, dstcol):
            """8-ki projection chain xs[:,:,xcol:xcol+width] -> dst cols."""
            which = "k" if w_s is wk_s else "q"
            written.setdefault((which, blk), set()).update(
                range(dstcol // 128, (dstcol + width) // 128))
            ps = smp.tile([128, 512], F32, name="pps", tag="sm")
            for ki in range(8):
                nc.tensor.matmul(
                    ps[:, 0:width],
                    lhsT=w_s[:, ki, blk * 128:(blk + 1) * 128],
                    rhs=xs[:, ki, xcol:xcol + width],
                    start=(ki == 0), stop=(ki == 7))
            nc.vector.tensor_copy(dst[:, dstcol:dstcol + width], ps[:, 0:width])

        def blk_tile(which, blk):
            tiles = kts if which == "k" else qts
            if blk not in tiles:
                tiles[blk] = ktq.tile([128, T], BF16,
                                      name=f"{which}t{blk}",
                                      tag=which)
            return tiles[blk]

        # K/Q re-load fills via the 2-deep c slot (blocks 1-3).
        fill_q = []

        def fill_dma(blk, which, e):
            src = xk if which == "k" else xq
            xs = load_eighth(src, e, nc.sync)
            fill_q.append((blk, which, e, xs))

        def fill_chain():
            blk, which, e, xs = fill_q.pop(0)
            w_s = wk_s if which == "k" else wq_s
            kq_chain(w_s, blk, xs, 0, 256, blk_tile(which, blk), e * 256)

        xv_q = [None] * 4

        def v_subq(h, qi):
            """V projection for head h, token blocks 4*qi..4*qi+4."""
            xs = xv_q[qi]
            for tb in range(4 * qi, 4 * qi + 4):
                ps = smp.tile([128, 512], F32, name="vps", tag="sm")
                for ki in range(8):
                    nc.tensor.matmul(
                        ps[:, 0:DK],
                        lhsT=xs[:, ki, (tb % 4) * 128:(tb % 4 + 1) * 128],
                        rhs=wv_s[:, ki, h * DK:(h + 1) * DK],
                        start=(ki == 0), stop=(ki == 7))
                nc.vector.tensor_copy(vaug[:, tb, h, 0:DK], ps[:, 0:DK])

        pts = {}

        def score_tile(n, kb, half=None):
            """Score matmuls + exp for tile (n, kb); half 0/1 = 512-wide."""
            h, qh = divmod(n, 2)
            blk, po = h // 2, (h % 2) * 64
            ktb, qtb = kts[blk], qts[blk]
            if (n, kb) not in pts:
                pts[(n, kb)] = ptp.tile([128, 1024], BF16,
                                        name="pt", tag="pt")
            pt = pts[(n, kb)]
            cs = (0, 1) if half is None else (half,)
            assert kb in written[("k", blk)], f"S({n},{kb}): kt{blk} race"
            for c in cs:
                need_q = range(8 * qh + 4 * c, 8 * qh + 4 * c + 4)
                assert written[("q", blk)].issuperset(need_q), \
                    f"S({n},{kb},{c}): qt{blk} race"
            st = bigp.tile([128, 1024], F32, name="st", tag="big")
            for c in cs:
                nc.tensor.matmul(
                    st[:, c * 512:(c + 1) * 512],
                    lhsT=ktb[po:po + 64, kb * 128:(kb + 1) * 128],
                    rhs=qtb[po:po + 64,
                            qh * 1024 + c * 512:qh * 1024 + (c + 1) * 512],
                    start=True, stop=True)
            c0, width = cs[0] * 512, len(cs) * 512
            nc.scalar.activation(
                pt[:, c0:c0 + width], st[:, c0:c0 + width],
                mybir.ActivationFunctionType.Exp, scale=EXP_SCALE)

        def pv_zero(accs=(0, 1)):
            """Zero PV accumulator banks with one start=True matmul each.
            PSUM zero-region granularity is the whole 2KB bank, so per-slot
            start=True flags from the interleaved qb series would re-mark
            each other's bytes and drop contributions; an explicit full-bank
            start + accumulate-only steps is the safe pattern."""
            for a in accs:
                acc = (pva, pvb)[a]
                nc.tensor.matmul(
                    acc.rearrange("p a b -> p (a b)"),
                    lhsT=zz[:, 0:128], rhs=zz[:, 0:512],
                    start=True, stop=False)

        def pv_step(n, kb, qbs):
            """PV accumulation of k-block kb for window n, given q-blocks."""
            h = n // 2
            if kb == 0 and n > 0 and qbs[0] == 0:
                pv_zero((0, 1))   # re-zero banks; norms(n-1) reads are done
            if qbs[-1] == 7:      # last reader of this pt tile
                pt = pts.pop((n, kb))
            else:
                pt = pts[(n, kb)]
            for qb in qbs:
                acc = (pva, pvb)[qb // 4]
                nc.tensor.matmul(
                    acc[:, qb % 4, 0:DK + 1],
                    lhsT=pt[:, qb * 128:(qb + 1) * 128],
                    rhs=vaug[:, kb, h, :],
                    start=False, stop=(kb == NKB - 1))

        pairs = {}

        def norms(n):
            """Normalize window n into pair tiles [128 q, 128 hd]. One
            batched reciprocal per bank (4 denominators), then 4 scalar
            multiplies; each bank is re-zeroed for the next window as soon
            as its four reads are done so the PE stall at the window
            boundary is short."""
            h, qh = divmod(n, 2)
            co = (h % 2) * 64
            for a, acc in enumerate((pva, pvb)):
                rd = nrm.tile([128, 4], F32, name="rd", tag="rd", bufs=2)
                nc.vector.reciprocal(rd[:], acc[:, :, DK:DK + 1])
                for s in range(4):
                    qb = 4 * a + s
                    if (qh, qb) not in pairs:
                        pairs[(qh, qb)] = prp.tile(
                            [128, 128], BF16, name=f"pr{qh}_{qb}",
                            tag=f"pr{qh}{qb}")
                    nc.vector.tensor_scalar_mul(
                        pairs[(qh, qb)][:, co:co + DK], acc[:, s, 0:DK],
                        rd[:, s:s + 1])

        tdone = set()

        def transposes(blk, qh, engs):
            """XBAR DMA transpose pair tiles -> otT for (blk, qh)."""
            tdone.add((blk, qh))
            for qb in range(8):
                pr = pairs.pop((qh, qb))
                engs[qb % len(engs)].dma_start_transpose(
                    otT[:, blk, qh * 1024 + qb * 128:qh * 1024 + (qb + 1) * 128],
                    pr)

        def fc_tb(tb, eng, use_smp=False):
            """Output projection for one token block (all 4 ki accumulated).
            Evictions rotate through dead staging slots. use_smp runs the
            two 512-col halves through the small PSUM pool so the score
            pipeline keeps both bigp slots during the last exp window."""
            ev_slots = ["s0", "s1", "s2", "s3", "wk", "wq", "c", "ev"]
            slot = ev_slots[tb % 8]
            pool = {"wk": wst, "wq": wst, "ev": evp}.get(slot, xst)
            ev = pool.tile([128, 1024], BF16, name="ev", tag=slot,
                           bufs=2 if slot == "c" else 1)
            if use_smp:
                for c in range(2):
                    fp = smp.tile([128, 512], F32, name="fch", tag="sm")
                    for ki in range(4):
                        nc.tensor.matmul(
                            fp[:],
                            lhsT=otT[:, ki, tb * 128:(tb + 1) * 128],
                            rhs=wot[:, ki, c * 512:(c + 1) * 512],
                            start=(ki == 0), stop=(ki == 3))
                    nc.vector.tensor_copy(ev[:, c * 512:(c + 1) * 512], fp[:])
            else:
                fp = bigp.tile([128, 1024], F32, name="fcp", tag="big")
                for c in range(2):
                    for ki in range(4):
                        nc.tensor.matmul(
                            fp[:, c * 512:(c + 1) * 512],
                            lhsT=otT[:, ki, tb * 128:(tb + 1) * 128],
                            rhs=wot[:, ki, c * 512:(c + 1) * 512],
                            start=(ki == 0), stop=(ki == 3))
                    nc.vector.tensor_copy(
                        ev[:, c * 512:(c + 1) * 512],
                        fp[:, c * 512:(c + 1) * 512])
                    eng.dma_start(
                        y[tb * 128:(tb + 1) * 128, c * 512:(c + 1) * 512],
                        ev[:, c * 512:(c + 1) * 512])
                return
            eng.dma_start(y[tb * 128:(tb + 1) * 128, :], ev[:])

        # ---- bootstrap ----
        # Queues: sync=SP, scalar=ACT, gpsimd=Pool(SWDGE). wq rides gpsimd,
        # wk + the xq eighths ride scalar, xk/xv quarters ride sync/gpsimd.
        nc.gpsimd.dma_start(wq_s[:, :, 0:256],
                            wq[:, 0:256].rearrange("(ki p) m -> p ki m", p=128))
        xqe = [load_eighth(xq, e, nc.scalar) for e in range(2)]
        nc.scalar.dma_start(wk_s[:, :, 0:256],
                            wk[:, 0:256].rearrange("(ki p) m -> p ki m", p=128))
        xk_q = [load_quarter(xk, 0, "s0", nc.sync)]

        # PE p-state warmup: the cost model runs the PE at 1.2 GHz until it
        # has been continuously busy for 3us. Junk matmuls bridge the DMA
        # wait so the real chains start at full speed (2.4 GHz).
        pv_zero()
        for j in range(5):
            jt = bigp.tile([128, 1024], F32, name="jnk", tag="big")
            nc.tensor.matmul(jt[:, 0:512], lhsT=zz[:, 0:128],
                             rhs=zz[:, 0:512], start=True, stop=True)

        blk_tile("k", 0)
        blk_tile("q", 0)

        # First chains: qt0 tokens 0:512 from the c-slot eighths (their DMAs
        # land first), then kt0 tokens 0:512 in 128-wide slices.
        kq_chain(wq_s, 0, xqe[0], 0, 256, qts[0], 0)
        kq_chain(wq_s, 0, xqe[1], 0, 256, qts[0], 256)
        for s in range(4):
            kq_chain(wk_s, 0, xk_q[0], s * 128, 128, kts[0], s * 128)
        nc.gpsimd.dma_start(wq_s[:, :, 256:512],
                            wq[:, 256:512].rearrange("(ki p) m -> p ki m", p=128))
        xqe += [load_eighth(xq, e, nc.scalar) for e in range(2, 4)]
        nc.scalar.dma_start(wk_s[:, :, 256:512],
                            wk[:, 256:512].rearrange("(ki p) m -> p ki m", p=128))
        nc.gpsimd.dma_start(wv_s, wv.rearrange("(ki p) m -> p ki m", p=128))
        xk_q.append(load_quarter(xk, 1, "s1", nc.sync))

        # First four score tiles in 512-wide halves to start the exp stream
        # while qt0's second half is still projecting.
        for kb in range(4):
            score_tile(0, kb, half=0)
        kq_chain(wq_s, 0, xqe[2], 0, 256, qts[0], 512)
        kq_chain(wq_s, 0, xqe[3], 0, 256, qts[0], 768)
        for kb in range(4):
            score_tile(0, kb, half=1)

        # ---- per-window work queues ----
        WORK = {w: [] for w in range(16)}

        def _xkq(i):
            def f():
                xk_q.append(load_quarter(xk, i, f"s{i}", nc.sync))
            return f

        def _ktchain(blk, i, dstcol):
            return lambda: kq_chain(wk_s, blk, xk_q[i], 0, 512,
                                    blk_tile("k", blk), dstcol)

        def _xqe(e):
            def f():
                xqe.append(load_eighth(xq, e, nc.scalar))
            return f

        def _qtchain(blk, e, dstcol):
            return lambda: kq_chain(wq_s, blk, xqe[e], 0, 256,
                                    blk_tile("q", blk), dstcol)

        def _xvq(i, eng):
            def f():
                xv_q[i] = load_quarter(xv, i, f"s{i}", eng)
            return f

        vdone = [0] * 8

        def _vsub(h, qi):
            def f():
                v_subq(h, qi)
                vdone[h] += 1
            return f

        def _wot():
            nc.gpsimd.dma_start(wot, wo.rearrange("(ki p) m -> p ki m", p=128))

        FILLS = ([(1, "k", e) for e in range(8)] +
                 [(1, "q", e) for e in range(8)] +
                 [(2, "k", e) for e in range(8)] +
                 [(2, "q", e) for e in range(8)] +
                 [(3, "k", e) for e in range(8)] +
                 [(3, "q", e) for e in range(8)])
        fill_i = [0]

        def _fill():
            blk, which, e = FILLS[fill_i[0]]
            fill_i[0] += 1
            fill_dma(blk, which, e)

        def _fc_half(tb, c, eng):
            """One 512-col half of FC for tb through the small PSUM pool;
            eviction + store on the c=1 half."""
            def f():
                ev_slots = ["s0", "s1", "s2", "s3", "wk", "wq", "c", "ev"]
                slot = ev_slots[tb % 8]
                pool = {"wk": wst, "wq": wst, "ev": evp}.get(slot, xst)
                if tb not in fc_ev:
                    fc_ev[tb] = pool.tile([128, 1024], BF16, name="ev",
                                          tag=slot,
                                          bufs=2 if slot == "c" else 1)
                ev = fc_ev[tb]
                fp = smp.tile([128, 512], F32, name="fch", tag="sm")
                for ki in range(4):
                    nc.tensor.matmul(
                        fp[:],
                        lhsT=otT[:, ki, tb * 128:(tb + 1) * 128],
                        rhs=wot[:, ki, c * 512:(c + 1) * 512],
                        start=(ki == 0), stop=(ki == 3))
                nc.vector.tensor_copy(ev[:, c * 512:(c + 1) * 512], fp[:])
                if c == 1:
                    eng.dma_start(y[tb * 128:(tb + 1) * 128, :], ev[:])
            return f

        fc_ev = {}
        fcp = {}

        def _fc2_part(tb):
            """ki0-2 partial of FC for token block tb (qh=1), to SBUF bf16.
            Runs during windows 12-14 so only the blk3 contraction and an
            add remain after the last exp."""
            def f():
                assert {(0, 1), (1, 1), (2, 1)} <= tdone, f"fc2 part {tb} race"
                fcp[tb] = fpp.tile([128, 1024], BF16, name=f"fcp{tb}",
                                   tag=f"fcp{tb}")
                for c in range(2):
                    fp = smp.tile([128, 512], F32, name="fc2p", tag="sm")
                    for ki in range(3):
                        nc.tensor.matmul(
                            fp[:],
                            lhsT=otT[:, ki, tb * 128:(tb + 1) * 128],
                            rhs=wot[:, ki, c * 512:(c + 1) * 512],
                            start=(ki == 0), stop=(ki == 2))
                    nc.vector.tensor_copy(
                        fcp[tb][:, c * 512:(c + 1) * 512], fp[:])
            return f

        def _fc2_fin(tb, eng):
            """Tail: blk3 contraction + add of the ki0-2 partial + store."""
            assert (3, 1) in tdone, f"fc2 fin {tb} race"
            ev_slots = ["s0", "s1", "s2", "s3", "wk", "wq", "c", "ev"]
            slot = ev_slots[tb % 8]
            pool = {"wk": wst, "wq": wst, "ev": evp}.get(slot, xst)
            ev = pool.tile([128, 1024], BF16, name="ev", tag=slot,
                           bufs=2 if slot == "c" else 1)
            for c in range(2):
                fp = smp.tile([128, 512], F32, name="fc2f", tag="sm")
                nc.tensor.matmul(
                    fp[:],
                    lhsT=otT[:, 3, tb * 128:(tb + 1) * 128],
                    rhs=wot[:, 3, c * 512:(c + 1) * 512],
                    start=True, stop=True)
                if tb >= 12:
                    # ACT (free after the exp stream) evicts the psum half;
                    # DVE then adds the bf16 partial in-place at 2x rate.
                    nc.scalar.copy(ev[:, c * 512:(c + 1) * 512], fp[:])
                    nc.vector.tensor_add(
                        ev[:, c * 512:(c + 1) * 512],
                        ev[:, c * 512:(c + 1) * 512],
                        fcp[tb][:, c * 512:(c + 1) * 512])
                else:
                    nc.vector.tensor_add(
                        ev[:, c * 512:(c + 1) * 512], fp[:],
                        fcp[tb][:, c * 512:(c + 1) * 512])
                eng.dma_start(
                    y[tb * 128:(tb + 1) * 128, c * 512:(c + 1) * 512],
                    ev[:, c * 512:(c + 1) * 512])

        # w0: rest of kt0 (tokens 512:2048 from xk quarters as they land),
        # qt0 tokens 1024:2048 (xq eighths 4-7), xv quarter loads, V0.
        # Order matters: the kt0 chain feeding score tiles (0, 4e..4e+4)
        # must be EMITTED before those score tiles (drain index < 4 * e + 4),
        # else the tile framework sees the read first (race -> garbage).
        WORK[0] = [
            _ktchain(0, 1, 512), _xkq(2),
            _xqe(4), _xvq(0, nc.gpsimd), _qtchain(0, 4, 1024),
            _xkq(3), _ktchain(0, 2, 1024), _vsub(0, 0),
            _xqe(5), _qtchain(0, 5, 1280),
            _xqe(6), _xvq(1, nc.gpsimd), _ktchain(0, 3, 1536),
            _qtchain(0, 6, 1536), _vsub(0, 1),
            _xqe(7), _qtchain(0, 7, 1792),
            _xvq(2, nc.gpsimd), _vsub(0, 2),
            _xvq(3, nc.gpsimd), _vsub(0, 3),
        ]
        # w1: kt1 e0-5 + V1 q0,q1; w2: qt1 e0-3 + kt1 e6,e7 + V1 q2,q3;
        # w3: qt1 e4-7 + V2. Fill order in FILLS is kt1, qt1, kt2/qt2,
        # kt3/qt3, so plain _fill/fill_chain pairs walk it. Then
        # w4-w12: block-2/3 fills, V3-V7 interleaved at odd windows.
        def fills(k):
            out = []
            for _ in range(k):
                out += [_fill, fill_chain]
            return out

        WORK[1] = (fills(2) + [_vsub(1, 0)] + fills(2) + [_vsub(1, 1)])
        WORK[2] = ([_wot] + fills(3) + [_vsub(1, 2)] + fills(3))
        WORK[3] = ([_vsub(1, 3)] + fills(1) + [_vsub(2, 0), _vsub(2, 1)] +
                   fills(1) + [_vsub(2, 2), _vsub(2, 3)])
        NFILL = {4: 5, 5: 5, 6: 5, 7: 5, 8: 4, 9: 4, 10: 4, 11: 4}
        VWIN = {5: (3, 0), 6: (3, 2), 7: (4, 0), 8: (4, 2), 9: (5, 0),
                10: (5, 2), 11: (6, 0), 12: (6, 2), 13: (7, 0), 14: (7, 2)}
        FC2P = {12: (8, 9, 10), 13: (11, 12), 14: (13, 14, 15)}
        for w in range(4, 15):
            items = []
            nf = NFILL.get(w, 0)
            for j in range(nf):
                items += [_fill, fill_chain]
            if w in VWIN:
                h, q0 = VWIN[w]
                vitems = [_vsub(h, q0), _vsub(h, q0 + 1)]
                merged = []
                for a, b in zip(items + [None] * 4, vitems + [None] * 8):
                    if a is not None:
                        merged.append(a)
                    if b is not None:
                        merged.append(b)
                items = merged
            items += [_fc2_part(tb) for tb in FC2P.get(w, ())]
            WORK[w] = items
        WORK[15] = []   # FC1 injected once transposes(3, 0) are emitted
        fc1_tail = []   # last FC1 halves run in the tail (PE idles there)

        # ---- steady state: entry list + trailing PV cursor ----
        entries = ([(0, kb, 0) for kb in range(4)] +
                   [(0, kb, 1) for kb in range(4)] +
                   [(0, kb, None) for kb in range(4, NKB)])
        for n in range(1, 16):
            entries += [(n, kb, None) for kb in range(NKB)]
        wstart = {n: (20 if n else 8) + 16 * (n - (0 if n == 0 else 1))
                  for n in range(16)}
        wcount = {n: (12 if n == 0 else 16) for n in range(16)}

        done = {w: 0 for w in range(16)}

        def drain(w, i, sub):
            for pw in range(w):        # flush leftovers of earlier windows
                lst = WORK.get(pw) or []
                while done[pw] < len(lst):
                    lst[done[pw]]()
                    done[pw] += 1
            lst = WORK.get(w)
            if not lst:
                return
            j = i - wstart[w]
            target = min(len(lst), (len(lst) * (2 * j + 1 + sub)
                                    + 2 * wcount[w] - 1) // (2 * wcount[w]))
            while done[w] < target:
                lst[done[w]]()
                done[w] += 1

        def pv_entry(idx, tail=False):
            n, kb, half = entries[idx]
            if vdone[n // 2] * 4 <= kb:   # V chains for this kb not emitted
                return False
            if half is None:
                qbs = range(8)
            else:
                qbs = range(4) if half == 0 else range(4, 8)
            pv_step(n, kb, qbs)
            if kb == NKB - 1 and (half is None or half == 1):
                norms(n)
                if n % 4 == 2:       # qh=0 half of block n//4 complete
                    transposes(n // 4, 0, [nc.sync])
                    if n == 14:
                        halves = [
                            _fc_half(tb, c, (nc.gpsimd, nc.sync)[tb % 2])
                            for tb in range(8) for c in range(2)]
                        WORK[15].extend(halves[:14])
                        fc1_tail.extend(halves[14:])
                elif n % 4 == 3:     # qh=1 half complete
                    engs = [nc.sync, nc.scalar] if tail else [nc.sync]
                    transposes(n // 4, 1, engs)
            return True

        def pv_lag(idx):
            # window-start pv entries carry the bank re-zero, which waits on
            # the previous window's norm reads (DVE); hold them back two
            # extra score tiles so the PE never idles on that wait.
            pn, pkb, phalf = entries[idx]
            return 4 if (pkb == 0 and pn > 0) else 2

        pv_cur = 0
        for i in range(8, len(entries)):
            n, kb, half = entries[i]
            drain(n, i, 0)
            score_tile(n, kb, half)
            while pv_cur <= i - pv_lag(pv_cur) and pv_entry(pv_cur):
                pv_cur += 1
            drain(n, i, 1)
        while pv_cur < len(entries):
            assert pv_entry(pv_cur, tail=True), "V chains missing at tail"
            pv_cur += 1
        for f in fc1_tail:   # fills PE during the blk3/qh1 transpose lead-in
            f()
        for tb in range(8, 16):
            _fc2_fin(tb, (nc.gpsimd, nc.sync, nc.scalar)[tb % 3])


_CACHED = None


def _build():
    global _CACHED
    if _CACHED is None:
        nc = bacc.Bacc("TRN2", target_bir_lowering=False, debug=False)
        _emit(nc)
        nc.compile()
        _CACHED = nc
    return _CACHED


def _run(inputs, trace=False, trace_kwargs=None):
    """Shard, run on 8 cores, gather. Returns (y, BassKernelResults)."""
    query, key, value = inputs["query"], inputs["key"], inputs["value"]
    Wq, Wk, Wv, Wo = inputs["Wq"], inputs["Wk"], inputs["Wv"], inputs["Wo"]
    bv, bo = inputs["bv"], inputs["bo"]

    f32 = np.float32
    wqT = np.asarray(Wq, f32).T.astype(NPBF16)   # [in, out]
    wkT = np.asarray(Wk, f32).T.astype(NPBF16)
    wvT = np.asarray(Wv, f32).T.astype(NPBF16)
    woT = np.asarray(Wo, f32).T.astype(NPBF16)   # [in(=hd), out]

    xqs = [np.asarray(query[b], f32).T.astype(NPBF16) for b in range(B)]
    xks = [np.asarray(key[b], f32).T.astype(NPBF16) for b in range(B)]
    xvs = [np.asarray(value[b], f32).T.astype(NPBF16) for b in range(B)]

    in_maps = []
    for c in range(NCORES):
        b, hh = divmod(c, 2)
        sl = slice(hh * DHALF, (hh + 1) * DHALF)
        in_maps.append({
            "xq": xqs[b], "xk": xks[b], "xv": xvs[b],
            "wq": np.ascontiguousarray(wqT[:, sl]),
            "wk": np.ascontiguousarray(wkT[:, sl]),
            "wv": np.ascontiguousarray(wvT[:, sl]),
            "wo": np.ascontiguousarray(woT[sl, :]),
        })

    nc = _build()
    kw = {}
    if trace:
        kw["trace"] = True
        kw["trace_kwargs"] = trace_kwargs or {}
    res = run_bass_kernel_spmd(nc, in_maps, core_ids=list(range(NCORES)), **kw)

    # host-side tensor-parallel reduction + exact bias
    bias = (np.asarray(bv, f32) @ np.asarray(Wo, f32).T + np.asarray(bo, f32))
    yout = np.empty((B, T, D), dtype=f32)
    for b in range(B):
        yout[b] = (np.asarray(res.results[2 * b]["y"], f32)
                   + np.asarray(res.results[2 * b + 1]["y"], f32))
        yout[b] += bias[None, :]
    return yout, res


def kernel(**inputs):
    yv, _ = _run(inputs, trace=False)
    return yv


# revision 50
# speedup vs baseline: 1.0128x; 1.0026x over previous
"""Multi-head attention (B=4, T=2048, D=1024, H=16) on 8 TRN2 NeuronCores.

Sharding: batch x head-half (4 batches x 2 halves of 8 heads = 8 cores).
Each core projects Q/K/V for its 8 heads over the full 2048 tokens, runs
attention, and computes partial output projections against its half of Wo.
The tensor-parallel FC "all-reduce" is a host-side sum of the partials.

Per-core program (all matmul inputs bf16, fp32 PSUM accumulation):
  - Scores S = K_blk @ Q^T land as [128 ktok, 1024 q] PSUM tiles; one exp
    per tile (scalar engine) writes P directly as bf16. The exp stream is
    the pacer (~266us); emission follows a global entry cursor: the score
    matmuls for entry i are gated on exp(i-2) freeing a PSUM slot, and the
    PV steps trail two entries behind on the same gates.
  - PV is kb-major and output-stationary: 8 PSUM accumulators [128 q, 65]
    (ones-augmented V gives the softmax denominator in col 64) accumulate
    one k-block right after its exp, so PV(n) completes with exp(n,15).
  - Normalization (DVE reciprocal + scalar multiply) writes two heads of a
    block side-by-side into [128 q, 128 hd] pair tiles; a DMA-engine XBAR
    transpose moves them into head-major otT with zero PE cost.
  - The FC accumulates all 4 head-pair blocks in PSUM per token block; the
    qh=0 half runs inside the last exp window, the qh=1 half trails PV(15).
  - Bootstrap: wq rides the gpsimd queue and the first xq eighths ride the
    small c-slots so the first exp fires ~9us in; the first four score
    tiles are split into 512-wide halves to keep the exp cursor moving
    while projections catch up. xk quarters hand their slots straight to
    xv (block-1 K/Q re-load as eighths in w1/w2, like the block-2/3 fills).

Host side: transposes inputs to feature-major bf16, slices weights per
head-half, runs SPMD on 8 cores, sums the two partial y per batch, and
adds the exact (bv @ Wo.T + bo) bias (attention rows sum to 1 so the value
bias passes through; bq/bk are zero in this problem).
"""
import numpy as np
from contextlib import ExitStack

import ml_dtypes

import concourse.bass as bass
import concourse.tile as tile
from concourse import bacc, mybir
from concourse.bass_utils import run_bass_kernel_spmd

F32 = mybir.dt.float32
BF16 = mybir.dt.bfloat16
NPBF16 = ml_dtypes.bfloat16

B = 4
T = 2048
D = 1024
H = 16
DK = 64
NCORES = 8
HLOC = 8           # heads per core
DHALF = 512        # hd dims per core
NKB = T // 128     # 16 key blocks
EXP_SCALE = 1.0 / np.sqrt(DK)


def _emit(nc):
    xq = nc.dram_tensor("xq", [D, T], BF16, kind="ExternalInput").ap()   # query^T
    xk = nc.dram_tensor("xk", [D, T], BF16, kind="ExternalInput").ap()   # key^T
    xv = nc.dram_tensor("xv", [D, T], BF16, kind="ExternalInput").ap()   # value^T
    wq = nc.dram_tensor("wq", [D, DHALF], BF16, kind="ExternalInput").ap()
    wk = nc.dram_tensor("wk", [D, DHALF], BF16, kind="ExternalInput").ap()
    wv = nc.dram_tensor("wv", [D, DHALF], BF16, kind="ExternalInput").ap()
    wo = nc.dram_tensor("wo", [DHALF, D], BF16, kind="ExternalInput").ap()
    y = nc.dram_tensor("y", [T, D], BF16, kind="ExternalOutput").ap()  # partial

    with tile.TileContext(nc) as tc, ExitStack() as ctx:
        res = ctx.enter_context(tc.tile_pool(name="res", bufs=1))
        otT = res.tile([128, 4, T], BF16)      # normalized attention out^T
        vaug = res.tile([128, NKB, HLOC, DK + 1], BF16)
        wot = res.tile([128, 4, D], BF16)      # Wo^T slice [(ki p) m -> p ki m]
        zz = res.tile([1, 512], BF16)          # zero row for PSUM-bank zeroing
        nc.vector.memset(vaug[:, :, :, DK:DK + 1], 1.0)
        nc.vector.memset(zz[:], 0.0)

        # K^T / Q^T blocks [128 hd, 2048 tok], two rotating slots per tag.
        ktq = ctx.enter_context(tc.tile_pool(name="ktq", bufs=2))
        kts, qts = {}, {}

        wst = ctx.enter_context(tc.tile_pool(name="wst", bufs=1))
        wk_s = wst.tile([128, 8, DHALF], BF16, name="wk_s", tag="wk")
        wq_s = wst.tile([128, 8, DHALF], BF16, name="wq_s", tag="wq")
        wv_s = wst.tile([128, 8, DHALF], BF16, name="wv_s", tag="wv")

        # Input staging: four quarter slots [128, 8, 512] carry xk then xv
        # (xv pinned for the per-head V quanta); slot c is a 2-deep eighth
        # slab [128, 8, 256] for all xq traffic and the K/Q re-load fills.
        xst = ctx.enter_context(tc.tile_pool(name="xst", bufs=1))

        # PSUM (8 banks): score/FC tiles [128,1024] x2 (4 banks), PV
        # accumulators [128,4,128] x2 (2), projection chains [128,512] x2.
        bigp = ctx.enter_context(tc.tile_pool(name="bigp", bufs=2, space="PSUM"))
        pvp = ctx.enter_context(tc.tile_pool(name="pvp", bufs=1, space="PSUM"))
        pva = pvp.tile([128, 4, 128], F32, name="pva", tag="pva")
        pvb = pvp.tile([128, 4, 128], F32, name="pvb", tag="pvb")
        smp = ctx.enter_context(tc.tile_pool(name="smp", bufs=2, space="PSUM"))

        ptp = ctx.enter_context(tc.tile_pool(name="ptp", bufs=14))  # P ring
        nrm = ctx.enter_context(tc.tile_pool(name="nrm", bufs=1))   # recip
        prp = ctx.enter_context(tc.tile_pool(name="prp", bufs=2))   # pair tiles
        evp = ctx.enter_context(tc.tile_pool(name="evp", bufs=1))   # fc evict
        fpp = ctx.enter_context(tc.tile_pool(name="fpp", bufs=1))   # fc2 partials

        def load_quarter(src, i, slot, eng):
            xs = xst.tile([128, 8, 512], BF16, name=f"x_{slot}", tag=slot)
            eng.dma_start(
                xs, src[:, i * 512:(i + 1) * 512]
                .rearrange("(ki p) t -> p ki t", p=128))
            return xs

        def load_eighth(src, e, eng):
            xs = xst.tile([128, 8, 256], BF16, name="x_c", tag="c", bufs=2)
            eng.dma_start(
                xs, src[:, e * 256:(e + 1) * 256]
                .rearrange("(ki p) t -> p ki t", p=128))
            return xs

        written = {}   # (which, blk) -> set of written 128-col chunks

        def kq_chain(w_s, blk, xs, xcol, width, dst, dstcol):
            """8-ki projection chain xs[:,:,xcol:xcol+width] -> dst cols."""
            which = "k" if w_s is wk_s else "q"
            written.setdefault((which, blk), set()).update(
                range(dstcol // 128, (dstcol + width) // 128))
            ps = smp.tile([128, 512], F32, name="pps", tag="sm")
            for ki in range(8):
                nc.tensor.matmul(
                    ps[:, 0:width],
                    lhsT=w_s[:, ki, blk * 128:(blk + 1) * 128],
                    rhs=xs[:, ki, xcol:xcol + width],
                    start=(ki == 0), stop=(ki == 7))
            nc.vector.tensor_copy(dst[:, dstcol:dstcol + width], ps[:, 0:width])

        def blk_tile(which, blk):
            tiles = kts if which == "k" else qts
            if blk not in tiles:
                tiles[blk] = ktq.tile([128, T], BF16,
                                      name=f"{which}t{blk}",
                                      tag=which)
            return tiles[blk]

        # K/Q re-load fills via the 2-deep c slot (blocks 1-3).
        fill_q = []

        def fill_dma(blk, which, e):
            src = xk if which == "k" else xq
            xs = load_eighth(src, e, nc.sync)
            fill_q.append((blk, which, e, xs))

        def fill_chain():
            blk, which, e, xs = fill_q.pop(0)
            w_s = wk_s if which == "k" else wq_s
            kq_chain(w_s, blk, xs, 0, 256, blk_tile(which, blk), e * 256)

        xv_q = [None] * 4

        def v_subq(h, qi):
            """V projection for head h, token blocks 4*qi..4*qi+4."""
            xs = xv_q[qi]
            for tb in range(4 * qi, 4 * qi + 4):
                ps = smp.tile([128, 512], F32, name="vps", tag="sm")
                for ki in range(8):
                    nc.tensor.matmul(
                        ps[:, 0:DK],
                        lhsT=xs[:, ki, (tb % 4) * 128:(tb % 4 + 1) * 128],
                        rhs=wv_s[:, ki, h * DK:(h + 1) * DK],
                        start=(ki == 0), stop=(ki == 7))
                nc.vector.tensor_copy(vaug[:, tb, h, 0:DK], ps[:, 0:DK])

        pts = {}

        def score_tile(n, kb, half=None):
            """Score matmuls + exp for tile (n, kb); half 0/1 = 512-wide."""
            h, qh = divmod(n, 2)
            blk, po = h // 2, (h % 2) * 64
            ktb, qtb = kts[blk], qts[blk]
            if (n, kb) not in pts:
                pts[(n, kb)] = ptp.tile([128, 1024], BF16,
                                        name="pt", tag="pt")
            pt = pts[(n, kb)]
            cs = (0, 1) if half is None else (half,)
            assert kb in written[("k", blk)], f"S({n},{kb}): kt{blk} race"
            for c in cs:
                need_q = range(8 * qh + 4 * c, 8 * qh + 4 * c + 4)
                assert written[("q", blk)].issuperset(need_q), \
                    f"S({n},{kb},{c}): qt{blk} race"
            st = bigp.tile([128, 1024], F32, name="st", tag="big")
            for c in cs:
                nc.tensor.matmul(
                    st[:, c * 512:(c + 1) * 512],
                    lhsT=ktb[po:po + 64, kb * 128:(kb + 1) * 128],
                    rhs=qtb[po:po + 64,
                            qh * 1024 + c * 512:qh * 1024 + (c + 1) * 512],
                    start=True, stop=True)
            c0, width = cs[0] * 512, len(cs) * 512
            nc.scalar.activation(
                pt[:, c0:c0 + width], st[:, c0:c0 + width],
                mybir.ActivationFunctionType.Exp, scale=EXP_SCALE)

        def pv_zero(accs=(0, 1)):
            """Zero PV accumulator banks with one start=True matmul each.
            PSUM zero-region granularity is the whole 2KB bank, so per-slot
            start=True flags from the interleaved qb series would re-mark
            each other's bytes and drop contributions; an explicit full-bank
            start + accumulate-only steps is the safe pattern."""
            for a in accs:
                acc = (pva, pvb)[a]
                nc.tensor.matmul(
                    acc.rearrange("p a b -> p (a b)"),
                    lhsT=zz[:, 0:128], rhs=zz[:, 0:512],
                    start=True, stop=False)

        def pv_step(n, kb, qbs):
            """PV accumulation of k-block kb for window n, given q-blocks."""
            h = n // 2
            if kb == 0 and n > 0 and qbs[0] == 0:
                pv_zero((0, 1))   # re-zero banks; norms(n-1) reads are done
            if qbs[-1] == 7:      # last reader of this pt tile
                pt = pts.pop((n, kb))
            else:
                pt = pts[(n, kb)]
            for qb in qbs:
                acc = (pva, pvb)[qb // 4]
                nc.tensor.matmul(
                    acc[:, qb % 4, 0:DK + 1],
                    lhsT=pt[:, qb * 128:(qb + 1) * 128],
                    rhs=vaug[:, kb, h, :],
                    start=False, stop=(kb == NKB - 1))

        pairs = {}

        def norms(n):
            """Normalize window n into pair tiles [128 q, 128 hd]. One
            batched reciprocal per bank (4 denominators), then 4 scalar
            multiplies; each bank is re-zeroed for the next window as soon
            as its four reads are done so the PE stall at the window
            boundary is short."""
            h, qh = divmod(n, 2)
            co = (h % 2) * 64
            for a, acc in enumerate((pva, pvb)):
                rd = nrm.tile([128, 4], F32, name="rd", tag="rd", bufs=2)
                nc.vector.reciprocal(rd[:], acc[:, :, DK:DK + 1])
                for s in range(4):
                    qb = 4 * a + s
                    if (qh, qb) not in pairs:
                        pairs[(qh, qb)] = prp.tile(
                            [128, 128], BF16, name=f"pr{qh}_{qb}",
                            tag=f"pr{qh}{qb}")
                    nc.vector.tensor_scalar_mul(
                        pairs[(qh, qb)][:, co:co + DK], acc[:, s, 0:DK],
                        rd[:, s:s + 1])

        tdone = set()

        def transposes(blk, qh, engs):
            """XBAR DMA transpose pair tiles -> otT for (blk, qh)."""
            tdone.add((blk, qh))
            for qb in range(8):
                pr = pairs.pop((qh, qb))
                engs[qb % len(engs)].dma_start_transpose(
                    otT[:, blk, qh * 1024 + qb * 128:qh * 1024 + (qb + 1) * 128],
                    pr)

        def fc_tb(tb, eng, use_smp=False):
            """Output projection for one token block (all 4 ki accumulated).
            Evictions rotate through dead staging slots. use_smp runs the
            two 512-col halves through the small PSUM pool so the score
            pipeline keeps both bigp slots during the last exp window."""
            ev_slots = ["s0", "s1", "s2", "s3", "wk", "wq", "c", "ev"]
            slot = ev_slots[tb % 8]
            pool = {"wk": wst, "wq": wst, "ev": evp}.get(slot, xst)
            ev = pool.tile([128, 1024], BF16, name="ev", tag=slot,
                           bufs=2 if slot == "c" else 1)
            if use_smp:
                for c in range(2):
                    fp = smp.tile([128, 512], F32, name="fch", tag="sm")
                    for ki in range(4):
                        nc.tensor.matmul(
                            fp[:],
                            lhsT=otT[:, ki, tb * 128:(tb + 1) * 128],
                            rhs=wot[:, ki, c * 512:(c + 1) * 512],
                            start=(ki == 0), stop=(ki == 3))
                    nc.vector.tensor_copy(ev[:, c * 512:(c + 1) * 512], fp[:])
            else:
                fp = bigp.tile([128, 1024], F32, name="fcp", tag="big")
                for c in range(2):
                    for ki in range(4):
                        nc.tensor.matmul(
                            fp[:, c * 512:(c + 1) * 512],
                            lhsT=otT[:, ki, tb * 128:(tb + 1) * 128],
                            rhs=wot[:, ki, c * 512:(c + 1) * 512],
                            start=(ki == 0), stop=(ki == 3))
                    nc.vector.tensor_copy(
                        ev[:, c * 512:(c + 1) * 512],
                        fp[:, c * 512:(c + 1) * 512])
                    eng.dma_start(
                        y[tb * 128:(tb + 1) * 128, c * 512:(c + 1) * 512],
                        ev[:, c * 512:(c + 1) * 512])
                return
            eng.dma_start(y[tb * 128:(tb + 1) * 128, :], ev[:])

        # ---- bootstrap ----
        # Queues: sync=SP, scalar=ACT, gpsimd=Pool(SWDGE). wq rides gpsimd,
        # wk + the xq eighths ride scalar, xk/xv quarters ride sync/gpsimd.
        nc.gpsimd.dma_start(wq_s[:, :, 0:256],
                            wq[:, 0:256].rearrange("(ki p) m -> p ki m", p=128))
        xqe = [load_eighth(xq, e, nc.scalar) for e in range(2)]
        nc.scalar.dma_start(wk_s[:, :, 0:256],
                            wk[:, 0:256].rearrange("(ki p) m -> p ki m", p=128))
        xk_q = [load_quarter(xk, 0, "s0", nc.sync)]

        # PE p-state warmup: the cost model runs the PE at 1.2 GHz until it
        # has been continuously busy for 3us. Junk matmuls bridge the DMA
        # wait so the real chains start at full speed (2.4 GHz).
        pv_zero()
        for j in range(5):
            jt = bigp.tile([128, 1024], F32, name="jnk", tag="big")
            nc.tensor.matmul(jt[:, 0:512], lhsT=zz[:, 0:128],
                             rhs=zz[:, 0:512], start=True, stop=True)

        blk_tile("k", 0)
        blk_tile("q", 0)

        # First chains: qt0 tokens 0:512 from the c-slot eighths (their DMAs
        # land first), then kt0 tokens 0:512 in 128-wide slices.
        kq_chain(wq_s, 0, xqe[0], 0, 256, qts[0], 0)
        kq_chain(wq_s, 0, xqe[1], 0, 256, qts[0], 256)
        for s in range(4):
            kq_chain(wk_s, 0, xk_q[0], s * 128, 128, kts[0], s * 128)
        nc.gpsimd.dma_start(wq_s[:, :, 256:512],
                            wq[:, 256:512].rearrange("(ki p) m -> p ki m", p=128))
        xqe += [load_eighth(xq, e, nc.scalar) for e in range(2, 4)]
        nc.scalar.dma_start(wk_s[:, :, 256:512],
                            wk[:, 256:512].rearrange("(ki p) m -> p ki m", p=128))
        nc.gpsimd.dma_start(wv_s, wv.rearrange("(ki p) m -> p ki m", p=128))
        xk_q.append(load_quarter(xk, 1, "s1", nc.sync))

        # First four score tiles in 512-wide halves to start the exp stream
        # while qt0's second half is still projecting.
        for kb in range(4):
            score_tile(0, kb, half=0)
        kq_chain(wq_s, 0, xqe[2], 0, 256, qts[0], 512)
        kq_chain(wq_s, 0, xqe[3], 0, 256, qts[0], 768)
        for kb in range(4):
            score_tile(0, kb, half=1)

        # ---- per-window work queues ----
        WORK = {w: [] for w in range(16)}

        def _xkq(i):
            def f():
                xk_q.append(load_quarter(xk, i, f"s{i}", nc.sync))
            return f

        def _ktchain(blk, i, dstcol):
            return lambda: kq_chain(wk_s, blk, xk_q[i], 0, 512,
                                    blk_tile("k", blk), dstcol)

        def _xqe(e):
            def f():
                xqe.append(load_eighth(xq, e, nc.scalar))
            return f

        def _qtchain(blk, e, dstcol):
            return lambda: kq_chain(wq_s, blk, xqe[e], 0, 256,
                                    blk_tile("q", blk), dstcol)

        def _xvq(i, eng):
            def f():
                xv_q[i] = load_quarter(xv, i, f"s{i}", eng)
            return f

        vdone = [0] * 8

        def _vsub(h, qi):
            def f():
                v_subq(h, qi)
                vdone[h] += 1
            return f

        def _wot():
            nc.gpsimd.dma_start(wot, wo.rearrange("(ki p) m -> p ki m", p=128))

        FILLS = ([(1, "k", e) for e in range(8)] +
                 [(1, "q", e) for e in range(8)] +
                 [(2, "k", e) for e in range(8)] +
                 [(2, "q", e) for e in range(8)] +
                 [(3, "k", e) for e in range(8)] +
                 [(3, "q", e) for e in range(8)])
        fill_i = [0]

        def _fill():
            blk, which, e = FILLS[fill_i[0]]
            fill_i[0] += 1
            fill_dma(blk, which, e)

        def _fc_half(tb, c, eng):
            """One 512-col half of FC for tb through the small PSUM pool;
            eviction + store on the c=1 half."""
            def f():
                ev_slots = ["s0", "s1", "s2", "s3", "wk", "wq", "c", "ev"]
                slot = ev_slots[tb % 8]
                pool = {"wk": wst, "wq": wst, "ev": evp}.get(slot, xst)
                if tb not in fc_ev:
                    fc_ev[tb] = pool.tile([128, 1024], BF16, name="ev",
                                          tag=slot,
                                          bufs=2 if slot == "c" else 1)
                ev = fc_ev[tb]
                fp = smp.tile([128, 512], F32, name="fch", tag="sm")
                for ki in range(4):
                    nc.tensor.matmul(
                        fp[:],
                        lhsT=otT[:, ki, tb * 128:(tb + 1) * 128],
                        rhs=wot[:, ki, c * 512:(c + 1) * 512],
                        start=(ki == 0), stop=(ki == 3))
                nc.vector.tensor_copy(ev[:, c * 512:(c + 1) * 512], fp[:])
                if c == 1:
                    eng.dma_start(y[tb * 128:(tb + 1) * 128, :], ev[:])
            return f

        fc_ev = {}
        fcp = {}

        def _fc2_part(tb):
            """ki0-2 partial of FC for token block tb (qh=1), to SBUF bf16.
            Runs during windows 12-14 so only the blk3 contraction and an
            add remain after the last exp."""
            def f():
                assert {(0, 1), (1, 1), (2, 1)} <= tdone, f"fc2 part {tb} race"
                fcp[tb] = fpp.tile([128, 1024], BF16, name=f"fcp{tb}",
                                   tag=f"fcp{tb}")
                for c in range(2):
                    fp = smp.tile([128, 512], F32, name="fc2p", tag="sm")
                    for ki in range(3):
                        nc.tensor.matmul(
                            fp[:],
                            lhsT=otT[:, ki, tb * 128:(tb + 1) * 128],
                            rhs=wot[:, ki, c * 512:(c + 1) * 512],
                            start=(ki == 0), stop=(ki == 2))
                    nc.vector.tensor_copy(
                        fcp[tb][:, c * 512:(c + 1) * 512], fp[:])
            return f

        def _fc2_fin(tb, eng):
            """Tail: blk3 contraction + add of the ki0-2 partial + store."""
            assert (3, 1) in tdone, f"fc2 fin {tb} race"
            ev_slots = ["s0", "s1", "s2", "s3", "wk", "wq", "c", "ev"]
            slot = ev_slots[tb % 8]
            pool = {"wk": wst, "wq": wst, "ev": evp}.get(slot, xst)
            ev = pool.tile([128, 1024], BF16, name="ev", tag=slot,
                           bufs=2 if slot == "c" else 1)
            for c in range(2):
                fp = smp.tile([128, 512], F32, name="fc2f", tag="sm")
                nc.tensor.matmul(
                    fp[:],
                    lhsT=otT[:, 3, tb * 128:(tb + 1) * 128],
                    rhs=wot[:, 3, c * 512:(c + 1) * 512],
                    start=True, stop=True)
                if tb >= 10:
                    # ACT (free after the exp stream) evicts the psum half;
                    # DVE then adds the bf16 partial in-place at 2x rate.
                    nc.scalar.copy(ev[:, c * 512:(c + 1) * 512], fp[:])
                    nc.vector.tensor_add(
                        ev[:, c * 512:(c + 1) * 512],
                        ev[:, c * 512:(c + 1) * 512],
                        fcp[tb][:, c * 512:(c + 1) * 512])
                else:
                    nc.vector.tensor_add(
                        ev[:, c * 512:(c + 1) * 512], fp[:],
                        fcp[tb][:, c * 512:(c + 1) * 512])
                eng.dma_start(
                    y[tb * 128:(tb + 1) * 128, c * 512:(c + 1) * 512],
                    ev[:, c * 512:(c + 1) * 512])

        # w0: rest of kt0 (tokens 512:2048 from xk quarters as they land),
        # qt0 tokens 1024:2048 (xq eighths 4-7), xv quarter loads, V0.
        # Order matters: the kt0 chain feeding score tiles (0, 4e..4e+4)
        # must be EMITTED before those score tiles (drain index < 4 * e + 4),
        # else the tile framework sees the read first (race -> garbage).
        WORK[0] = [
            _ktchain(0, 1, 512), _xkq(2),
            _xqe(4), _xvq(0, nc.gpsimd), _qtchain(0, 4, 1024),
            _xkq(3), _ktchain(0, 2, 1024), _vsub(0, 0),
            _xqe(5), _qtchain(0, 5, 1280),
            _xqe(6), _xvq(1, nc.gpsimd), _ktchain(0, 3, 1536),
            _qtchain(0, 6, 1536), _vsub(0, 1),
            _xqe(7), _qtchain(0, 7, 1792),
            _xvq(2, nc.gpsimd), _vsub(0, 2),
            _xvq(3, nc.gpsimd), _vsub(0, 3),
        ]
        # w1: kt1 e0-5 + V1 q0,q1; w2: qt1 e0-3 + kt1 e6,e7 + V1 q2,q3;
        # w3: qt1 e4-7 + V2. Fill order in FILLS is kt1, qt1, kt2/qt2,
        # kt3/qt3, so plain _fill/fill_chain pairs walk it. Then
        # w4-w12: block-2/3 fills, V3-V7 interleaved at odd windows.
        def fills(k):
            out = []
            for _ in range(k):
                out += [_fill, fill_chain]
            return out

        WORK[1] = (fills(2) + [_vsub(1, 0)] + fills(2) + [_vsub(1, 1)])
        WORK[2] = ([_wot] + fills(3) + [_vsub(1, 2)] + fills(3))
        WORK[3] = ([_vsub(1, 3)] + fills(1) + [_vsub(2, 0), _vsub(2, 1)] +
                   fills(1) + [_vsub(2, 2), _vsub(2, 3)])
        NFILL = {4: 5, 5: 5, 6: 5, 7: 5, 8: 4, 9: 4, 10: 4, 11: 4}
        VWIN = {5: (3, 0), 6: (3, 2), 7: (4, 0), 8: (4, 2), 9: (5, 0),
                10: (5, 2), 11: (6, 0), 12: (6, 2), 13: (7, 0), 14: (7, 2)}
        FC2P = {12: (8, 9, 10), 13: (11, 12, 13), 14: (14, 15)}
        for w in range(4, 15):
            items = []
            nf = NFILL.get(w, 0)
            for j in range(nf):
                items += [_fill, fill_chain]
            if w in VWIN:
                h, q0 = VWIN[w]
                vitems = [_vsub(h, q0), _vsub(h, q0 + 1)]
                merged = []
                for a, b in zip(items + [None] * 4, vitems + [None] * 8):
                    if a is not None:
                        merged.append(a)
                    if b is not None:
                        merged.append(b)
                items = merged
            items += [_fc2_part(tb) for tb in FC2P.get(w, ())]
            WORK[w] = items
        WORK[15] = []   # FC1 injected once transposes(3, 0) are emitted
        fc1_tail = []   # last FC1 halves run in the tail (PE idles there)

        # ---- steady state: entry list + trailing PV cursor ----
        entries = ([(0, kb, 0) for kb in range(4)] +
                   [(0, kb, 1) for kb in range(4)] +
                   [(0, kb, None) for kb in range(4, NKB)])
        for n in range(1, 16):
            entries += [(n, kb, None) for kb in range(NKB)]
        wstart = {n: (20 if n else 8) + 16 * (n - (0 if n == 0 else 1))
                  for n in range(16)}
        wcount = {n: (12 if n == 0 else 16) for n in range(16)}

        done = {w: 0 for w in range(16)}

        def drain(w, i, sub):
            for pw in range(w):        # flush leftovers of earlier windows
                lst = WORK.get(pw) or []
                while done[pw] < len(lst):
                    lst[done[pw]]()
                    done[pw] += 1
            lst = WORK.get(w)
            if not lst:
                return
            j = i - wstart[w]
            target = min(len(lst), (len(lst) * (2 * j + 1 + sub)
                                    + 2 * wcount[w] - 1) // (2 * wcount[w]))
            while done[w] < target:
                lst[done[w]]()
                done[w] += 1

        def pv_entry(idx, tail=False):
            n, kb, half = entries[idx]
            if vdone[n // 2] * 4 <= kb:   # V chains for this kb not emitted
                return False
            if half is None:
                qbs = range(8)
            else:
                qbs = range(4) if half == 0 else range(4, 8)
            pv_step(n, kb, qbs)
            if kb == NKB - 1 and (half is None or half == 1):
                norms(n)
                if n % 4 == 2:       # qh=0 half of block n//4 complete
                    transposes(n // 4, 0, [nc.sync])
                    if n == 14:
                        halves = [
                            _fc_half(tb, c, (nc.gpsimd, nc.sync)[tb % 2])
                            for tb in range(8) for c in range(2)]
                        WORK[15].extend(halves[:14])
                        fc1_tail.extend(halves[14:])
                elif n % 4 == 3:     # qh=1 half complete
                    engs = [nc.sync, nc.scalar] if tail else [nc.sync]
                    transposes(n // 4, 1, engs)
            return True

        def pv_lag(idx):
            # window-start pv entries carry the bank re-zero, which waits on
            # the previous window's norm reads (DVE); hold them back two
            # extra score tiles so the PE never idles on that wait.
            pn, pkb, phalf = entries[idx]
            return 4 if (pkb == 0 and pn > 0) else 2

        pv_cur = 0
        for i in range(8, len(entries)):
            n, kb, half = entries[i]
            drain(n, i, 0)
            score_tile(n, kb, half)
            while pv_cur <= i - pv_lag(pv_cur) and pv_entry(pv_cur):
                pv_cur += 1
            drain(n, i, 1)
        while pv_cur < len(entries):
            assert pv_entry(pv_cur, tail=True), "V chains missing at tail"
            pv_cur += 1
        for f in fc1_tail:   # fills PE during the blk3/qh1 transpose lead-in
            f()
        for tb in (10, 8, 11, 9, 12, 13, 14, 15):
            _fc2_fin(tb, (nc.gpsimd, nc.sync)[tb % 2])


_CACHED = None


def _build():
    global _CACHED
    if _CACHED is None:
        nc = bacc.Bacc("TRN2", target_bir_lowering=False, debug=False)
        _emit(nc)
        nc.compile()
        _CACHED = nc
    return _CACHED


def _run(inputs, trace=False, trace_kwargs=None):
    """Shard, run on 8 cores, gather. Returns (y, BassKernelResults)."""
    query, key, value = inputs["query"], inputs["key"], inputs["value"]
    Wq, Wk, Wv, Wo = inputs["Wq"], inputs["Wk"], inputs["Wv"], inputs["Wo"]
    bv, bo = inputs["bv"], inputs["bo"]

    f32 = np.float32
    wqT = np.asarray(Wq, f32).T.astype(NPBF16)   # [in, out]
    wkT = np.asarray(Wk, f32).T.astype(NPBF16)
    wvT = np.asarray(Wv, f32).T.astype(NPBF16)
    woT = np.asarray(Wo, f32).T.astype(NPBF16)   # [in(=hd), out]

    xqs = [np.asarray(query[b], f32).T.astype(NPBF16) for b in range(B)]
    xks = [np.asarray(key[b], f32).T.astype(NPBF16) for b in range(B)]
    xvs = [np.asarray(value[b], f32).T.astype(NPBF16) for b in range(B)]

    in_maps = []
    for c in range(NCORES):
        b, hh = divmod(c, 2)
        sl = slice(hh * DHALF, (hh + 1) * DHALF)
        in_maps.append({
            "xq": xqs[b], "xk": xks[b], "xv": xvs[b],
            "wq": np.ascontiguousarray(wqT[:, sl]),
            "wk": np.ascontiguousarray(wkT[:, sl]),
            "wv": np.ascontiguousarray(wvT[:, sl]),
            "wo": np.ascontiguousarray(woT[sl, :]),
        })

    nc = _build()
    kw = {}
    if trace:
        kw["trace"] = True
        kw["trace_kwargs"] = trace_kwargs or {}
    res = run_bass_kernel_spmd(nc, in_maps, core_ids=list(range(NCORES)), **kw)

    # host-side tensor-parallel reduction + exact bias
    bias = (np.asarray(bv, f32) @ np.asarray(Wo, f32).T + np.asarray(bo, f32))
    yout = np.empty((B, T, D), dtype=f32)
    for b in range(B):
        yout[b] = (np.asarray(res.results[2 * b]["y"], f32)
                   + np.asarray(res.results[2 * b + 1]["y"], f32))
        yout[b] += bias[None, :]
    return yout, res


def kernel(**inputs):
    yv, _ = _run(inputs, trace=False)
    return yv


# revision 51
# speedup vs baseline: 1.0235x; 1.0106x over previous
"""Multi-head attention (B=4, T=2048, D=1024, H=16) on 8 TRN2 NeuronCores.

Sharding: batch x head-half (4 batches x 2 halves of 8 heads = 8 cores).
Each core projects Q/K/V for its 8 heads over the full 2048 tokens, runs
attention, and computes partial output projections against its half of Wo.
The tensor-parallel FC "all-reduce" is a host-side sum of the partials.

Per-core program (all matmul inputs bf16, fp32 PSUM accumulation):
  - Scores S = K_blk @ Q^T land as [128 ktok, 1024 q] PSUM tiles; one exp
    per tile (scalar engine) writes P directly as bf16. The exp stream is
    the pacer (~266us); emission follows a global entry cursor: the score
    matmuls for entry i are gated on exp(i-2) freeing a PSUM slot, and the
    PV steps trail two entries behind on the same gates.
  - PV is kb-major and output-stationary: 8 PSUM accumulators [128 q, 65]
    (ones-augmented V gives the softmax denominator in col 64) accumulate
    one k-block right after its exp, so PV(n) completes with exp(n,15).
  - Normalization (DVE reciprocal + scalar multiply) writes two heads of a
    block side-by-side into [128 q, 128 hd] pair tiles; a DMA-engine XBAR
    transpose moves them into head-major otT with zero PE cost.
  - The FC accumulates all 4 head-pair blocks in PSUM per token block; the
    qh=0 half runs inside the last exp window, the qh=1 half trails PV(15).
  - Bootstrap: wq rides the gpsimd queue and the first xq eighths ride the
    small c-slots so the first exp fires ~9us in; the first four score
    tiles are split into 512-wide halves to keep the exp cursor moving
    while projections catch up. xk quarters hand their slots straight to
    xv (block-1 K/Q re-load as eighths in w1/w2, like the block-2/3 fills).

Host side: transposes inputs to feature-major bf16, slices weights per
head-half, runs SPMD on 8 cores, sums the two partial y per batch, and
adds the exact (bv @ Wo.T + bo) bias (attention rows sum to 1 so the value
bias passes through; bq/bk are zero in this problem).
"""
import numpy as np
from contextlib import ExitStack

import ml_dtypes

import concourse.bass as bass
import concourse.tile as tile
from concourse import bacc, mybir
from concourse.bass_utils import run_bass_kernel_spmd

F32 = mybir.dt.float32
BF16 = mybir.dt.bfloat16
NPBF16 = ml_dtypes.bfloat16

B = 4
T = 2048
D = 1024
H = 16
DK = 64
NCORES = 8
HLOC = 8           # heads per core
DHALF = 512        # hd dims per core
NKB = T // 128     # 16 key blocks
EXP_SCALE = 1.0 / np.sqrt(DK)


def _emit(nc):
    xq = nc.dram_tensor("xq", [D, T], BF16, kind="ExternalInput").ap()   # query^T
    xk = nc.dram_tensor("xk", [D, T], BF16, kind="ExternalInput").ap()   # key^T
    xv = nc.dram_tensor("xv", [D, T], BF16, kind="ExternalInput").ap()   # value^T
    wq = nc.dram_tensor("wq", [D, DHALF], BF16, kind="ExternalInput").ap()
    wk = nc.dram_tensor("wk", [D, DHALF], BF16, kind="ExternalInput").ap()
    wv = nc.dram_tensor("wv", [D, DHALF], BF16, kind="ExternalInput").ap()
    wo = nc.dram_tensor("wo", [DHALF, D], BF16, kind="ExternalInput").ap()
    y = nc.dram_tensor("y", [T, D], BF16, kind="ExternalOutput").ap()  # partial

    with tile.TileContext(nc) as tc, ExitStack() as ctx:
        res = ctx.enter_context(tc.tile_pool(name="res", bufs=1))
        otT = res.tile([128, 4, T], BF16)      # normalized attention out^T
        vaug = res.tile([128, NKB, HLOC, DK + 1], BF16)
        wot = res.tile([128, 4, D], BF16)      # Wo^T slice [(ki p) m -> p ki m]
        zz = res.tile([1, 512], BF16)          # zero row for PSUM-bank zeroing
        nc.vector.memset(vaug[:, :, :, DK:DK + 1], 1.0)
        nc.vector.memset(zz[:], 0.0)

        # K^T / Q^T blocks [128 hd, 2048 tok], two rotating slots per tag.
        ktq = ctx.enter_context(tc.tile_pool(name="ktq", bufs=2))
        kts, qts = {}, {}

        wst = ctx.enter_context(tc.tile_pool(name="wst", bufs=1))
        wk_s = wst.tile([128, 8, DHALF], BF16, name="wk_s", tag="wk")
        wq_s = wst.tile([128, 8, DHALF], BF16, name="wq_s", tag="wq")
        wv_s = wst.tile([128, 8, DHALF], BF16, name="wv_s", tag="wv")

        # Input staging: four quarter slots [128, 8, 512] carry xk then xv
        # (xv pinned for the per-head V quanta); slot c is a 2-deep eighth
        # slab [128, 8, 256] for all xq traffic and the K/Q re-load fills.
        xst = ctx.enter_context(tc.tile_pool(name="xst", bufs=1))

        # PSUM (8 banks): score/FC tiles [128,1024] x2 (4 banks), PV
        # accumulators [128,4,128] x2 (2), projection chains [128,512] x2.
        bigp = ctx.enter_context(tc.tile_pool(name="bigp", bufs=2, space="PSUM"))
        pvp = ctx.enter_context(tc.tile_pool(name="pvp", bufs=1, space="PSUM"))
        pva = pvp.tile([128, 4, 128], F32, name="pva", tag="pva")
        pvb = pvp.tile([128, 4, 128], F32, name="pvb", tag="pvb")
        smp = ctx.enter_context(tc.tile_pool(name="smp", bufs=2, space="PSUM"))

        ptp = ctx.enter_context(tc.tile_pool(name="ptp", bufs=14))  # P ring
        nrm = ctx.enter_context(tc.tile_pool(name="nrm", bufs=1))   # recip
        prp = ctx.enter_context(tc.tile_pool(name="prp", bufs=2))   # pair tiles
        evp = ctx.enter_context(tc.tile_pool(name="evp", bufs=1))   # fc evict
        fpp = ctx.enter_context(tc.tile_pool(name="fpp", bufs=1))   # fc2 partials

        def load_quarter(src, i, slot, eng):
            xs = xst.tile([128, 8, 512], BF16, name=f"x_{slot}", tag=slot)
            eng.dma_start(
                xs, src[:, i * 512:(i + 1) * 512]
                .rearrange("(ki p) t -> p ki t", p=128))
            return xs

        def load_eighth(src, e, eng):
            xs = xst.tile([128, 8, 256], BF16, name="x_c", tag="c", bufs=2)
            eng.dma_start(
                xs, src[:, e * 256:(e + 1) * 256]
                .rearrange("(ki p) t -> p ki t", p=128))
            return xs

        written = {}   # (which, blk) -> set of written 128-col chunks

        def kq_chain(w_s, blk, xs, xcol, width, dst, dstcol):
            """8-ki projection chain xs[:,:,xcol:xcol+width] -> dst cols."""
            which = "k" if w_s is wk_s else "q"
            written.setdefault((which, blk), set()).update(
                range(dstcol // 128, (dstcol + width) // 128))
            ps = smp.tile([128, 512], F32, name="pps", tag="sm")
            for ki in range(8):
                nc.tensor.matmul(
                    ps[:, 0:width],
                    lhsT=w_s[:, ki, blk * 128:(blk + 1) * 128],
                    rhs=xs[:, ki, xcol:xcol + width],
                    start=(ki == 0), stop=(ki == 7))
            nc.vector.tensor_copy(dst[:, dstcol:dstcol + width], ps[:, 0:width])

        def blk_tile(which, blk):
            tiles = kts if which == "k" else qts
            if blk not in tiles:
                tiles[blk] = ktq.tile([128, T], BF16,
                                      name=f"{which}t{blk}",
                                      tag=which)
            return tiles[blk]

        # K/Q re-load fills via the 2-deep c slot (blocks 1-3).
        fill_q = []

        def fill_dma(blk, which, e):
            src = xk if which == "k" else xq
            xs = load_eighth(src, e, nc.sync)
            fill_q.append((blk, which, e, xs))

        def fill_chain():
            blk, which, e, xs = fill_q.pop(0)
            w_s = wk_s if which == "k" else wq_s
            kq_chain(w_s, blk, xs, 0, 256, blk_tile(which, blk), e * 256)

        xv_q = [None] * 4

        def v_subq(h, qi):
            """V projection for head h, token blocks 4*qi..4*qi+4."""
            xs = xv_q[qi]
            for tb in range(4 * qi, 4 * qi + 4):
                ps = smp.tile([128, 512], F32, name="vps", tag="sm")
                for ki in range(8):
                    nc.tensor.matmul(
                        ps[:, 0:DK],
                        lhsT=xs[:, ki, (tb % 4) * 128:(tb % 4 + 1) * 128],
                        rhs=wv_s[:, ki, h * DK:(h + 1) * DK],
                        start=(ki == 0), stop=(ki == 7))
                nc.vector.tensor_copy(vaug[:, tb, h, 0:DK], ps[:, 0:DK])

        pts = {}

        def score_tile(n, kb, half=None):
            """Score matmuls + exp for tile (n, kb); half 0/1 = 512-wide."""
            h, qh = divmod(n, 2)
            blk, po = h // 2, (h % 2) * 64
            ktb, qtb = kts[blk], qts[blk]
            if (n, kb) not in pts:
                pts[(n, kb)] = ptp.tile([128, 1024], BF16,
                                        name="pt", tag="pt")
            pt = pts[(n, kb)]
            cs = (0, 1) if half is None else (half,)
            assert kb in written[("k", blk)], f"S({n},{kb}): kt{blk} race"
            for c in cs:
                need_q = range(8 * qh + 4 * c, 8 * qh + 4 * c + 4)
                assert written[("q", blk)].issuperset(need_q), \
                    f"S({n},{kb},{c}): qt{blk} race"
            st = bigp.tile([128, 1024], F32, name="st", tag="big")
            for c in cs:
                nc.tensor.matmul(
                    st[:, c * 512:(c + 1) * 512],
                    lhsT=ktb[po:po + 64, kb * 128:(kb + 1) * 128],
                    rhs=qtb[po:po + 64,
                            qh * 1024 + c * 512:qh * 1024 + (c + 1) * 512],
                    start=True, stop=True)
            c0, width = cs[0] * 512, len(cs) * 512
            nc.scalar.activation(
                pt[:, c0:c0 + width], st[:, c0:c0 + width],
                mybir.ActivationFunctionType.Exp, scale=EXP_SCALE)

        def pv_zero(accs=(0, 1)):
            """Zero PV accumulator banks with one start=True matmul each.
            PSUM zero-region granularity is the whole 2KB bank, so per-slot
            start=True flags from the interleaved qb series would re-mark
            each other's bytes and drop contributions; an explicit full-bank
            start + accumulate-only steps is the safe pattern."""
            for a in accs:
                acc = (pva, pvb)[a]
                nc.tensor.matmul(
                    acc.rearrange("p a b -> p (a b)"),
                    lhsT=zz[:, 0:128], rhs=zz[:, 0:512],
                    start=True, stop=False)

        def pv_step(n, kb, qbs):
            """PV accumulation of k-block kb for window n, given q-blocks."""
            h = n // 2
            if kb == 0 and n > 0 and qbs[0] == 0:
                pv_zero((0, 1))   # re-zero banks; norms(n-1) reads are done
            if qbs[-1] == 7:      # last reader of this pt tile
                pt = pts.pop((n, kb))
            else:
                pt = pts[(n, kb)]
            for qb in qbs:
                acc = (pva, pvb)[qb // 4]
                nc.tensor.matmul(
                    acc[:, qb % 4, 0:DK + 1],
                    lhsT=pt[:, qb * 128:(qb + 1) * 128],
                    rhs=vaug[:, kb, h, :],
                    start=False, stop=(kb == NKB - 1))

        pairs = {}

        def norms(n):
            """Normalize window n into pair tiles [128 q, 128 hd]. One
            batched reciprocal per bank (4 denominators), then 4 scalar
            multiplies; each bank is re-zeroed for the next window as soon
            as its four reads are done so the PE stall at the window
            boundary is short."""
            h, qh = divmod(n, 2)
            co = (h % 2) * 64
            for a, acc in enumerate((pva, pvb)):
                rd = nrm.tile([128, 4], F32, name="rd", tag="rd", bufs=2)
                nc.vector.reciprocal(rd[:], acc[:, :, DK:DK + 1])
                for s in range(4):
                    qb = 4 * a + s
                    if (qh, qb) not in pairs:
                        pairs[(qh, qb)] = prp.tile(
                            [128, 128], BF16, name=f"pr{qh}_{qb}",
                            tag=f"pr{qh}{qb}")
                    nc.vector.tensor_scalar_mul(
                        pairs[(qh, qb)][:, co:co + DK], acc[:, s, 0:DK],
                        rd[:, s:s + 1])

        tdone = set()

        def transposes(blk, qh, engs):
            """XBAR DMA transpose pair tiles -> otT for (blk, qh)."""
            tdone.add((blk, qh))
            for qb in range(8):
                pr = pairs.pop((qh, qb))
                engs[qb % len(engs)].dma_start_transpose(
                    otT[:, blk, qh * 1024 + qb * 128:qh * 1024 + (qb + 1) * 128],
                    pr)

        def fc_tb(tb, eng, use_smp=False):
            """Output projection for one token block (all 4 ki accumulated).
            Evictions rotate through dead staging slots. use_smp runs the
            two 512-col halves through the small PSUM pool so the score
            pipeline keeps both bigp slots during the last exp window."""
            ev_slots = ["s0", "s1", "s2", "s3", "wk", "wq", "c", "ev"]
            slot = ev_slots[tb % 8]
            pool = {"wk": wst, "wq": wst, "ev": evp}.get(slot, xst)
            ev = pool.tile([128, 1024], BF16, name="ev", tag=slot,
                           bufs=2 if slot == "c" else 1)
            if use_smp:
                for c in range(2):
                    fp = smp.tile([128, 512], F32, name="fch", tag="sm")
                    for ki in range(4):
                        nc.tensor.matmul(
                            fp[:],
                            lhsT=otT[:, ki, tb * 128:(tb + 1) * 128],
                            rhs=wot[:, ki, c * 512:(c + 1) * 512],
                            start=(ki == 0), stop=(ki == 3))
                    nc.vector.tensor_copy(ev[:, c * 512:(c + 1) * 512], fp[:])
            else:
                fp = bigp.tile([128, 1024], F32, name="fcp", tag="big")
                for c in range(2):
                    for ki in range(4):
                        nc.tensor.matmul(
                            fp[:, c * 512:(c + 1) * 512],
                            lhsT=otT[:, ki, tb * 128:(tb + 1) * 128],
                            rhs=wot[:, ki, c * 512:(c + 1) * 512],
                            start=(ki == 0), stop=(ki == 3))
                    nc.vector.tensor_copy(
                        ev[:, c * 512:(c + 1) * 512],
                        fp[:, c * 512:(c + 1) * 512])
                    eng.dma_start(
                        y[tb * 128:(tb + 1) * 128, c * 512:(c + 1) * 512],
                        ev[:, c * 512:(c + 1) * 512])
                return
            eng.dma_start(y[tb * 128:(tb + 1) * 128, :], ev[:])

        # ---- bootstrap ----
        # Queues: sync=SP, scalar=ACT, gpsimd=Pool(SWDGE). wq rides gpsimd,
        # wk + the xq eighths ride scalar, xk/xv quarters ride sync/gpsimd.
        nc.gpsimd.dma_start(wq_s[:, :, 0:256],
                            wq[:, 0:256].rearrange("(ki p) m -> p ki m", p=128))
        xqe = [load_eighth(xq, e, nc.scalar) for e in range(2)]
        nc.scalar.dma_start(wk_s[:, :, 0:256],
                            wk[:, 0:256].rearrange("(ki p) m -> p ki m", p=128))
        xk_q = [load_quarter(xk, 0, "s0", nc.sync)]

        # PE p-state warmup: the cost model runs the PE at 1.2 GHz until it
        # has been continuously busy for 3us. Junk matmuls bridge the DMA
        # wait so the real chains start at full speed (2.4 GHz).
        pv_zero()
        for j in range(5):
            jt = bigp.tile([128, 1024], F32, name="jnk", tag="big")
            nc.tensor.matmul(jt[:, 0:512], lhsT=zz[:, 0:128],
                             rhs=zz[:, 0:512], start=True, stop=True)

        blk_tile("k", 0)
        blk_tile("q", 0)

        # First chains: qt0 tokens 0:512 from the c-slot eighths (their DMAs
        # land first), then kt0 tokens 0:512 in 128-wide slices.
        kq_chain(wq_s, 0, xqe[0], 0, 256, qts[0], 0)
        kq_chain(wq_s, 0, xqe[1], 0, 256, qts[0], 256)
        for s in range(4):
            kq_chain(wk_s, 0, xk_q[0], s * 128, 128, kts[0], s * 128)
        nc.gpsimd.dma_start(wq_s[:, :, 256:512],
                            wq[:, 256:512].rearrange("(ki p) m -> p ki m", p=128))
        xqe += [load_eighth(xq, e, nc.scalar) for e in range(2, 4)]
        nc.scalar.dma_start(wk_s[:, :, 256:512],
                            wk[:, 256:512].rearrange("(ki p) m -> p ki m", p=128))
        nc.gpsimd.dma_start(wv_s, wv.rearrange("(ki p) m -> p ki m", p=128))
        xk_q.append(load_quarter(xk, 1, "s1", nc.sync))

        # First four score tiles in 512-wide halves to start the exp stream
        # while qt0's second half is still projecting.
        for kb in range(4):
            score_tile(0, kb, half=0)
        kq_chain(wq_s, 0, xqe[2], 0, 256, qts[0], 512)
        kq_chain(wq_s, 0, xqe[3], 0, 256, qts[0], 768)
        for kb in range(4):
            score_tile(0, kb, half=1)

        # ---- per-window work queues ----
        WORK = {w: [] for w in range(16)}

        def _xkq(i):
            def f():
                xk_q.append(load_quarter(xk, i, f"s{i}", nc.sync))
            return f

        def _ktchain(blk, i, dstcol):
            return lambda: kq_chain(wk_s, blk, xk_q[i], 0, 512,
                                    blk_tile("k", blk), dstcol)

        def _xqe(e):
            def f():
                xqe.append(load_eighth(xq, e, nc.scalar))
            return f

        def _qtchain(blk, e, dstcol):
            return lambda: kq_chain(wq_s, blk, xqe[e], 0, 256,
                                    blk_tile("q", blk), dstcol)

        def _xvq(i, eng):
            def f():
                xv_q[i] = load_quarter(xv, i, f"s{i}", eng)
            return f

        vdone = [0] * 8

        def _vsub(h, qi):
            def f():
                v_subq(h, qi)
                vdone[h] += 1
            return f

        def _wot():
            nc.gpsimd.dma_start(wot, wo.rearrange("(ki p) m -> p ki m", p=128))

        FILLS = ([(1, "k", e) for e in range(8)] +
                 [(1, "q", e) for e in range(8)] +
                 [(2, "k", e) for e in range(8)] +
                 [(2, "q", e) for e in range(8)] +
                 [(3, "k", e) for e in range(8)] +
                 [(3, "q", e) for e in range(8)])
        fill_i = [0]

        def _fill():
            blk, which, e = FILLS[fill_i[0]]
            fill_i[0] += 1
            fill_dma(blk, which, e)

        def _fc_half(tb, c, eng):
            """One 512-col half of FC for tb through the small PSUM pool;
            eviction + store on the c=1 half."""
            def f():
                ev_slots = ["s0", "s1", "s2", "s3", "wk", "wq", "c", "ev"]
                slot = ev_slots[tb % 8]
                pool = {"wk": wst, "wq": wst, "ev": evp}.get(slot, xst)
                if tb not in fc_ev:
                    fc_ev[tb] = pool.tile([128, 1024], BF16, name="ev",
                                          tag=slot,
                                          bufs=2 if slot == "c" else 1)
                ev = fc_ev[tb]
                fp = smp.tile([128, 512], F32, name="fch", tag="sm")
                for ki in range(4):
                    nc.tensor.matmul(
                        fp[:],
                        lhsT=otT[:, ki, tb * 128:(tb + 1) * 128],
                        rhs=wot[:, ki, c * 512:(c + 1) * 512],
                        start=(ki == 0), stop=(ki == 3))
                nc.vector.tensor_copy(ev[:, c * 512:(c + 1) * 512], fp[:])
                if c == 1:
                    eng.dma_start(y[tb * 128:(tb + 1) * 128, :], ev[:])
            return f

        fc_ev = {}
        fcp = {}

        def _fc2_part(tb):
            """ki0-2 partial of FC for token block tb, to SBUF bf16: only
            the blk3 contraction and an add remain after its gating norm."""
            def f():
                qh = tb // 8
                assert {(0, qh), (1, qh), (2, qh)} <= tdone, \
                    f"fc part {tb} race"
                fcp[tb] = fpp.tile([128, 1024], BF16, name=f"fcp{tb}",
                                   tag=f"fcp{tb}")
                for c in range(2):
                    fp = smp.tile([128, 512], F32, name="fc2p", tag="sm")
                    for ki in range(3):
                        nc.tensor.matmul(
                            fp[:],
                            lhsT=otT[:, ki, tb * 128:(tb + 1) * 128],
                            rhs=wot[:, ki, c * 512:(c + 1) * 512],
                            start=(ki == 0), stop=(ki == 2))
                    nc.vector.tensor_copy(
                        fcp[tb][:, c * 512:(c + 1) * 512], fp[:])
            return f

        def _fc2_fin(tb, eng, act_assist=False):
            """blk3 contraction + add of the ki0-2 partial + store."""
            assert (3, tb // 8) in tdone, f"fc fin {tb} race"
            ev_slots = ["s0", "s1", "s2", "s3", "wk", "wq", "c", "ev"]
            slot = ev_slots[tb % 8]
            pool = {"wk": wst, "wq": wst, "ev": evp}.get(slot, xst)
            ev = pool.tile([128, 1024], BF16, name="ev", tag=slot,
                           bufs=2 if slot == "c" else 1)
            for c in range(2):
                fp = smp.tile([128, 512], F32, name="fc2f", tag="sm")
                nc.tensor.matmul(
                    fp[:],
                    lhsT=otT[:, 3, tb * 128:(tb + 1) * 128],
                    rhs=wot[:, 3, c * 512:(c + 1) * 512],
                    start=True, stop=True)
                if act_assist:
                    # ACT (free after the exp stream) evicts the psum half;
                    # DVE then adds the bf16 partial in-place at 2x rate.
                    nc.scalar.copy(ev[:, c * 512:(c + 1) * 512], fp[:])
                    nc.vector.tensor_add(
                        ev[:, c * 512:(c + 1) * 512],
                        ev[:, c * 512:(c + 1) * 512],
                        fcp[tb][:, c * 512:(c + 1) * 512])
                else:
                    nc.vector.tensor_add(
                        ev[:, c * 512:(c + 1) * 512], fp[:],
                        fcp[tb][:, c * 512:(c + 1) * 512])
                eng.dma_start(
                    y[tb * 128:(tb + 1) * 128, c * 512:(c + 1) * 512],
                    ev[:, c * 512:(c + 1) * 512])

        # w0: rest of kt0 (tokens 512:2048 from xk quarters as they land),
        # qt0 tokens 1024:2048 (xq eighths 4-7), xv quarter loads, V0.
        # Order matters: the kt0 chain feeding score tiles (0, 4e..4e+4)
        # must be EMITTED before those score tiles (drain index < 4 * e + 4),
        # else the tile framework sees the read first (race -> garbage).
        WORK[0] = [
            _ktchain(0, 1, 512), _xkq(2),
            _xqe(4), _xvq(0, nc.gpsimd), _qtchain(0, 4, 1024),
            _xkq(3), _ktchain(0, 2, 1024), _vsub(0, 0),
            _xqe(5), _qtchain(0, 5, 1280),
            _xqe(6), _xvq(1, nc.gpsimd), _ktchain(0, 3, 1536),
            _qtchain(0, 6, 1536), _vsub(0, 1),
            _xqe(7), _qtchain(0, 7, 1792),
            _xvq(2, nc.gpsimd), _vsub(0, 2),
            _xvq(3, nc.gpsimd), _vsub(0, 3),
        ]
        # w1: kt1 e0-5 + V1 q0,q1; w2: qt1 e0-3 + kt1 e6,e7 + V1 q2,q3;
        # w3: qt1 e4-7 + V2. Fill order in FILLS is kt1, qt1, kt2/qt2,
        # kt3/qt3, so plain _fill/fill_chain pairs walk it. Then
        # w4-w12: block-2/3 fills, V3-V7 interleaved at odd windows.
        def fills(k):
            out = []
            for _ in range(k):
                out += [_fill, fill_chain]
            return out

        WORK[1] = (fills(2) + [_vsub(1, 0)] + fills(2) + [_vsub(1, 1)])
        WORK[2] = ([_wot] + fills(3) + [_vsub(1, 2)] + fills(3))
        WORK[3] = ([_vsub(1, 3)] + fills(1) + [_vsub(2, 0), _vsub(2, 1)] +
                   fills(1) + [_vsub(2, 2), _vsub(2, 3)])
        NFILL = {4: 5, 5: 5, 6: 5, 7: 5, 8: 5, 9: 5, 10: 5, 11: 1}
        VWIN = {5: (3, 0), 6: (3, 2), 7: (4, 0), 8: (4, 2), 9: (5, 0),
                10: (5, 2)}
        FCP = {11: (0, 1), 12: (2, 3), 13: (4, 5, 6, 7), 14: (8, 9, 10, 11),
               15: (12, 13, 14, 15)}
        for w in range(4, 15):
            items = []
            nf = NFILL.get(w, 0)
            for j in range(nf):
                items += [_fill, fill_chain]
            if w in VWIN:
                h, q0 = VWIN[w]
                vitems = [_vsub(h, q0), _vsub(h, q0 + 1)]
                merged = []
                for a, b in zip(items + [None] * 4, vitems + [None] * 8):
                    if a is not None:
                        merged.append(a)
                    if b is not None:
                        merged.append(b)
                items = merged
            if w == 11:
                items += [_vsub(6, qi) for qi in range(4)]
            if w == 12:
                items += [_vsub(7, qi) for qi in range(4)]
            items += [_fc2_part(tb) for tb in FCP.get(w, ())]
            WORK[w] = items
        WORK[15] = [_fc2_part(tb) for tb in FCP.get(15, ())]
        # FC1 fins appended once transposes(3, 0) are emitted (n=14 trail)

        # ---- steady state: entry list + trailing PV cursor ----
        entries = ([(0, kb, 0) for kb in range(4)] +
                   [(0, kb, 1) for kb in range(4)] +
                   [(0, kb, None) for kb in range(4, NKB)])
        for n in range(1, 16):
            entries += [(n, kb, None) for kb in range(NKB)]
        wstart = {n: (20 if n else 8) + 16 * (n - (0 if n == 0 else 1))
                  for n in range(16)}
        wcount = {n: (12 if n == 0 else 16) for n in range(16)}

        done = {w: 0 for w in range(16)}

        def drain(w, i, sub):
            for pw in range(w):        # flush leftovers of earlier windows
                lst = WORK.get(pw) or []
                while done[pw] < len(lst):
                    lst[done[pw]]()
                    done[pw] += 1
            lst = WORK.get(w)
            if not lst:
                return
            j = i - wstart[w]
            target = min(len(lst), (len(lst) * (2 * j + 1 + sub)
                                    + 2 * wcount[w] - 1) // (2 * wcount[w]))
            while done[w] < target:
                lst[done[w]]()
                done[w] += 1

        def pv_entry(idx, tail=False):
            n, kb, half = entries[idx]
            if vdone[n // 2] * 4 <= kb:   # V chains for this kb not emitted
                return False
            if half is None:
                qbs = range(8)
            else:
                qbs = range(4) if half == 0 else range(4, 8)
            pv_step(n, kb, qbs)
            if kb == NKB - 1 and (half is None or half == 1):
                norms(n)
                if n % 4 == 2:       # qh=0 half of block n//4 complete
                    transposes(n // 4, 0, [nc.sync])
                    if n == 14:
                        WORK[15].extend(
                            (lambda t: lambda: _fc2_fin(
                                t, (nc.gpsimd, nc.sync)[t % 2]))(tb)
                            for tb in range(8))
                elif n % 4 == 3:     # qh=1 half complete
                    engs = [nc.sync, nc.scalar] if tail else [nc.sync]
                    transposes(n // 4, 1, engs)
            return True

        def pv_lag(idx):
            # window-start pv entries carry the bank re-zero, which waits on
            # the previous window's norm reads (DVE); hold them back two
            # extra score tiles so the PE never idles on that wait.
            pn, pkb, phalf = entries[idx]
            return 4 if (pkb == 0 and pn > 0) else 2

        pv_cur = 0
        for i in range(8, len(entries)):
            n, kb, half = entries[i]
            drain(n, i, 0)
            score_tile(n, kb, half)
            while pv_cur <= i - pv_lag(pv_cur) and pv_entry(pv_cur):
                pv_cur += 1
            drain(n, i, 1)
        while pv_cur < len(entries):
            assert pv_entry(pv_cur, tail=True), "V chains missing at tail"
            pv_cur += 1
        for tb in (10, 8, 11, 9, 12, 13, 14, 15):
            _fc2_fin(tb, (nc.gpsimd, nc.sync)[tb % 2], act_assist=(tb >= 10))


_CACHED = None


def _build():
    global _CACHED
    if _CACHED is None:
        nc = bacc.Bacc("TRN2", target_bir_lowering=False, debug=False)
        _emit(nc)
        nc.compile()
        _CACHED = nc
    return _CACHED


def _run(inputs, trace=False, trace_kwargs=None):
    """Shard, run on 8 cores, gather. Returns (y, BassKernelResults)."""
    query, key, value = inputs["query"], inputs["key"], inputs["value"]
    Wq, Wk, Wv, Wo = inputs["Wq"], inputs["Wk"], inputs["Wv"], inputs["Wo"]
    bv, bo = inputs["bv"], inputs["bo"]

    f32 = np.float32
    wqT = np.asarray(Wq, f32).T.astype(NPBF16)   # [in, out]
    wkT = np.asarray(Wk, f32).T.astype(NPBF16)
    wvT = np.asarray(Wv, f32).T.astype(NPBF16)
    woT = np.asarray(Wo, f32).T.astype(NPBF16)   # [in(=hd), out]

    xqs = [np.asarray(query[b], f32).T.astype(NPBF16) for b in range(B)]
    xks = [np.asarray(key[b], f32).T.astype(NPBF16) for b in range(B)]
    xvs = [np.asarray(value[b], f32).T.astype(NPBF16) for b in range(B)]

    in_maps = []
    for c in range(NCORES):
        b, hh = divmod(c, 2)
        sl = slice(hh * DHALF, (hh + 1) * DHALF)
        in_maps.append({
            "xq": xqs[b], "xk": xks[b], "xv": xvs[b],
            "wq": np.ascontiguousarray(wqT[:, sl]),
            "wk": np.ascontiguousarray(wkT[:, sl]),
            "wv": np.ascontiguousarray(wvT[:, sl]),
            "wo": np.ascontiguousarray(woT[sl, :]),
        })

    nc = _build()
    kw = {}
    if trace:
        kw["trace"] = True
        kw["trace_kwargs"] = trace_kwargs or {}
    res = run_bass_kernel_spmd(nc, in_maps, core_ids=list(range(NCORES)), **kw)

    # host-side tensor-parallel reduction + exact bias
    bias = (np.asarray(bv, f32) @ np.asarray(Wo, f32).T + np.asarray(bo, f32))
    yout = np.empty((B, T, D), dtype=f32)
    for b in range(B):
        yout[b] = (np.asarray(res.results[2 * b]["y"], f32)
                   + np.asarray(res.results[2 * b + 1]["y"], f32))
        yout[b] += bias[None, :]
    return yout, res


def kernel(**inputs):
    yv, _ = _run(inputs, trace=False)
    return yv
